# revision 85
# baseline (speedup 1.0000x reference)
"""AttentionBlock3D on 8 Trainium2 NeuronCores — Gram-matrix restructure.

Math (see reference.py): GroupNorm(8 groups) -> qkv 1x1 conv -> channel
attention (contract over tokens N, softmax over last d=64) -> out proj ->
residual.

Sharding: N = T*H*W = 16384 tokens split 8 ways (2048/core); every core
holds all 4 batches of its token slice.  Cross-core state: one GroupNorm
stats AllReduce (16 KB, all batches merged) + one logits AllReduce per
batch (128 KB each).

Key identity: the channel-attention logits contract over tokens, so
  L = Q K^T = Aq Gx Ak^T + uq bk~^T + bq~ uk^T + N bq~ bk~^T
with Gx = sum_n x x^T (Gram of RAW x — independent of GroupNorm stats!),
Aq = Wq diag(a), uq = Aq Sx, bq~ = Wq c + bq (a, c = GN scale/shift).
Gx is computed on the PE from t~4us (PE-transposed x tiles), entirely
overlapping the stats pipeline + AllReduce; the remaining per-batch work
(Y = Gx Ak^T, L = Aq Y, rank-1 corrections) is tiny.  Per-core PE work
drops from ~300us (baseline QK-projection path) to ~240us and there is
no startup bubble.

Phases (per-core):
  A: x streamed once ([128 x 2048] tiles, 4 DMA queues); bn_stats (DVE)
     + Identity/Square accum (ACT) -> per-channel sum/sumsq; ONE stats
     AllReduce for all batches; PE meanwhile transposes x blocks and
     accumulates Gx(b) per batch in PSUM.
  B: per batch: aqkT = diag(a)*[Wq|Wk]^T (DVE row-scale), Y = Gx aqkT_k,
     L = aqkT_q^T Y (+ rank-1 bias/GN corrections in fp32), extract the 8
     diagonal 64x64 head blocks -> per-batch logits AllReduce.
  C: per batch: softmax + PE blockdiag-transpose, then per 512-token
     chunk: h = a*x+c (DVE), V proj, attn@v, out proj, +residual, store.

All big matmuls run in float32r (free dim 512 -> 1 cycle/row); the rank-1
correction matmuls and transposes are exact fp32.
"""

import numpy as np

import concourse.bass as bass
import concourse.mybir as mybir
import concourse.tile as tile
from concourse import bass_utils

F32 = mybir.dt.float32
F32R = mybir.dt.float32r
AX = mybir.AxisListType.X
ALU = mybir.AluOpType
ACT = mybir.ActivationFunctionType

N_CORES = 8
B, C, T, H, W = 4, 512, 16, 32, 32
N_TOT = T * H * W            # 16384
NH, D = 8, 64                # heads, head dim
G = 8                        # groupnorm groups
EPS = 1e-5
P = 128
CO = C // P                  # 4 channel chunks
NC = 512                     # phase-C token chunk size


def _round_tf32(a: np.ndarray) -> np.ndarray:
    """Round fp32 to fp32r (keep 10 explicit mantissa bits, RNE)."""
    u = a.astype(np.float32).view(np.uint32).astype(np.uint64)
    u = (u + 0x1000 + ((u >> 13) & 1)) & 0xFFFFE000
    return u.astype(np.uint32).view(np.float32)


def build_module(n_loc: int, debug: bool = False):
    NT = n_loc // P              # token blocks per batch (16)
    nchunks = n_loc // NC        # phase-C chunks per batch (4)
    ngr = n_loc // 512           # bn_stats groups per tile
    ntot = n_loc * N_CORES
    m_group = (C // G) * ntot    # elements per (b, group) stat
    scale = float(D) ** -0.5

    nc = bass.Bass("TRN2", target_bir_lowering=False, debug=False,
                   num_devices=N_CORES)

    xin = nc.dram_tensor("xin", [B, C, n_loc], F32, kind="ExternalInput").ap()
    wqk_t = nc.dram_tensor("wqk_t", [C, 2 * C], F32, kind="ExternalInput").ap()
    wv_t = nc.dram_tensor("wv_t", [C, C], F32, kind="ExternalInput").ap()
    wo_t = nc.dram_tensor("wo_t", [C, C], F32, kind="ExternalInput").ap()
    qkb = nc.dram_tensor("qkb", [1, 2 * C], F32, kind="ExternalInput").ap()
    vb2 = nc.dram_tensor("vb2", [P, CO], F32, kind="ExternalInput").ap()
    ob2 = nc.dram_tensor("ob2", [P, CO], F32, kind="ExternalInput").ap()
    gnw2 = nc.dram_tensor("gnw2", [P, CO], F32, kind="ExternalInput").ap()
    gnb2 = nc.dram_tensor("gnb2", [P, CO], F32, kind="ExternalInput").ap()
    yout = nc.dram_tensor("yout", [B, C, n_loc], F32, kind="ExternalOutput").ap()
    dbg = {}
    if debug:
        for nm, shp in [("dbg_stats", [P, 2, CO, B]), ("dbg_a", [P, CO, B]),
                        ("dbg_c", [P, CO, B]), ("dbg_gx", [P, CO, C]),
                        ("dbg_logits", [P, B, 4, D]),
                        ("dbg_attn", [P, B, 4, D]),
                        ("dbg_vec", [2, B, 2 * C]),
                        ("dbg_v", [P, CO, NC]), ("dbg_av", [P, CO, NC])]:
            dbg[nm] = nc.dram_tensor(nm, shp, F32, kind="ExternalOutput").ap()

    from concourse.masks import make_identity
    from bass_rust import add_dep_helper as _adh

    with tile.TileContext(nc) as tc:
        with (
            tc.tile_pool(name="persist", bufs=1) as pers,
            tc.tile_pool(name="wvpool", bufs=1) as wvp,
            tc.tile_pool(name="dram", bufs=1, space="DRAM") as dram,
        ):
            # ------------- persistent tiles (consts on gpsimd queue so
            # the HWDGE queues start streaming x at t=0) -------------
            qkb_sb = pers.tile([1, 2 * C], F32)
            vb_sb = pers.tile([P, CO], F32)
            ob_sb = pers.tile([P, CO], F32)
            gnw_sb = pers.tile([P, CO], F32)
            gnb_sb = pers.tile([P, CO], F32)

            ident = pers.tile([P, P], F32)
            make_identity(nc, ident[:])
            sel_sb = pers.tile([P, 2], F32)
            nc.vector.memset(sel_sb[:], 0.0)
            nc.vector.memset(sel_sb[0:64, 0:1], 1.0)
            nc.vector.memset(sel_sb[64:128, 1:2], 1.0)
            selt_sb = pers.tile([2, P], F32)
            with tc.tile_pool(name="selps", bufs=1, space="PSUM") as selps:
                sel_pt = selps.tile([2, P], F32)
                nc.tensor.transpose(sel_pt[:], sel_sb[:], ident[:])
                nc.vector.tensor_copy(selt_sb[:], sel_pt[:])

            a_sb = pers.tile([P, CO, B], F32)     # GN scale per (ci,co,b)
            c_sb = pers.tile([P, CO, B], F32)     # GN shift
            logits_sb = pers.tile([P, B, 4, D], F32)
            lg_full = pers.tile([P, B, 4, D], F32)

            stats = pers.tile([P, 2, CO, B], F32)   # local sum/sumsq
            stg = pers.tile([P, 2, CO, B], F32)     # global (post-AR)
            bst = pers.tile([P, B, CO, ngr, 6], F32)
            st_in = dram.tile([P, 2, CO, B], F32, name="st_in")
            st_gout = dram.tile([N_CORES, P, 2, CO, B], F32, name="st_gout")
            lg_in_l = [dram.tile([P, 2, 4, D], F32, name=f"lg_in{pp}")
                       for pp in range(B // 2)]
            lg_out_l = [dram.tile([P, 2, 4, D], F32, name=f"lg_out{pp}")
                        for pp in range(B // 2)]

            # rank-1 correction vectors (uqk/8, bqk, N*bqk/8) per batch, in
            # bf16 (the corrections are small relative to the logits, and
            # bf16 halves partition-0 SBUF pressure + runs 1 cycle/row)
            BF16 = mybir.dt.bfloat16
            cvec = pers.tile([1, 3, B, 2 * C], BF16)
            uqk8_l = [cvec[0:1, 0, bb] for bb in range(B)]
            bqk_l = [cvec[0:1, 1, bb] for bb in range(B)]
            nbq8_l = [cvec[0:1, 2, bb] for bb in range(B)]

            eps_t = pers.tile([2, 1], F32)
            nc.vector.memset(eps_t[:], EPS)

            # ================= phase A/B =================
            import contextlib
            ab_stack = contextlib.ExitStack()
            xp = ab_stack.enter_context(tc.tile_pool(name="xp", bufs=3))
            xs3p = ab_stack.enter_context(tc.tile_pool(name="xs3p", bufs=2))
            xtp = ab_stack.enter_context(tc.tile_pool(name="xtp", bufs=3))
            wqp = ab_stack.enter_context(tc.tile_pool(name="wqp", bufs=1))
            gxsb = ab_stack.enter_context(tc.tile_pool(name="gxsb", bufs=3))
            ysb = ab_stack.enter_context(tc.tile_pool(name="ysb", bufs=1))
            small = ab_stack.enter_context(tc.tile_pool(name="small", bufs=1))
            tps = ab_stack.enter_context(
                tc.tile_pool(name="tps", bufs=2, space="PSUM"))
            gxps = ab_stack.enter_context(
                tc.tile_pool(name="gxps", bufs=1, space="PSUM"))
            ylps = ab_stack.enter_context(
                tc.tile_pool(name="ylps", bufs=2, space="PSUM"))

            # x is loaded twice in phase A, as [P, 2, *] co-PAIR tiles (one
            # DMA per pair — per-DMA overhead is ~2.2us so bigger is better)
            # on the two HWDGE queues (SP: co 0/1, ACT: co 2/3); the gpsimd
            # queue stays free so the stats collective fires immediately:
            #  - transpose feed: [P, 2, 1024] halves (PE-paced)
            #  - stats feed: [P, 2, 2048], consumed right away (DVE
            #    bn_stats for co 0/1, ACT accum passes for co 2/3)
            qpair = [(nc.sync, 0), (nc.scalar, 2)]
            NJ = 2                      # transpose halves per (b, pair)
            gh = ngr // NJ              # bn_stats windows per half per co
            xa_t = {}

            def load_t(b, stats_here=True):
                # single x pass: [P, 2, 1024] co-pair halves feed BOTH the
                # PE transposes and (for b<3) the DVE bn_stats
                w = n_loc // NJ
                for jj in range(NJ):
                    for q, (eng, cb) in enumerate(qpair):
                        xa = xp.tile([P, 2, w], F32, tag=f"xt{q}",
                                     name=f"xt{b}_{q}_{jj}")
                        eng.dma_start(
                            xa[:],
                            xin[b, cb * P:(cb + 2) * P, jj * w:(jj + 1) * w]
                            .rearrange("(co ci) n -> ci co n", ci=P))
                        xa_t[(b, q, jj)] = xa
                        if stats_here:
                            for cc in range(2):
                                for g in range(gh):
                                    nc.vector.bn_stats(
                                        bst[:, b, cb + cc, jj * gh + g],
                                        xa[:, cc, g * 512:(g + 1) * 512])
                if stats_here:
                    stats_aggr(b)

            def stats_aggr(b):
                for co in range(CO):
                    mvt = small.tile([P, 2], F32, tag="mvt")
                    nc.vector.bn_aggr(mvt[:], bst[:, b, co])
                    nc.vector.tensor_scalar_mul(
                        stats[:, 0, co, b:b + 1], mvt[:, 0:1], float(n_loc))
                    nc.vector.tensor_tensor(
                        stats[:, 1, co, b:b + 1], mvt[:, 0:1],
                        mvt[:, 0:1], ALU.mult)
                    nc.vector.tensor_tensor(
                        stats[:, 1, co, b:b + 1],
                        stats[:, 1, co, b:b + 1], mvt[:, 1:2],
                        ALU.add)
                    nc.vector.tensor_scalar_mul(
                        stats[:, 1, co, b:b + 1],
                        stats[:, 1, co, b:b + 1], float(n_loc))

            # ---- PE: transpose x + accumulate Gram, software-pipelined
            # with lag so the ACT evacuation never stalls the PE ----
            TGX_LAG = 2
            gx_ps = {}
            gsb_l = {}

            def _emit_t(b, tb):
                tpb = NT // NJ          # tok-blocks per transpose half
                pt = tps.tile([P, C], F32, tag="t")
                for co in range(CO):
                    q, cc = divmod(co, 2)
                    nc.tensor.transpose(
                        pt[:, co * P:(co + 1) * P],
                        xa_t[(b, q, tb // tpb)][:, cc,
                                                (tb % tpb) * P:
                                                (tb % tpb + 1) * P],
                        ident[:])
                xt = xtp.tile([P, C], F32R, tag="xt")
                nc.scalar.copy(xt[:], pt[:])
                return xt

            def _emit_gx(b, tb, xt):
                if tb == 0:
                    gx_ps[b] = [gxps.tile([P, C], F32, tag=f"gx{co}",
                                          name=f"gx{b}_{co}")
                                for co in range(CO)]
                for co in range(CO):
                    nc.tensor.matmul(
                        gx_ps[b][co][:], xt[:, co * P:(co + 1) * P],
                        xt[:], start=(tb == 0), stop=(tb == NT - 1))
                if tb == NT - 1:
                    # evacuate Gram to SBUF (ACT — DVE runs bn_stats and
                    # must not serialize the Gram pipeline behind them)
                    gsb = gxsb.tile([P, CO, C], F32R, tag="gx",
                                    name=f"gxsb{b}")
                    for co in range(CO):
                        nc.scalar.copy(gsb[:, co], gx_ps[b][co][:])
                    if debug and b == 0:
                        nc.gpsimd.dma_start(dbg["dbg_gx"][:], gsb[:])
                    gsb_l[b] = gsb

            def t_gx_run(batches):
                steps = [(b, tb) for b in batches for tb in range(NT)]
                xts = {}
                for i, (b, tb) in enumerate(steps):
                    xts[i] = _emit_t(b, tb)
                    if i >= TGX_LAG:
                        bb, tt = steps[i - TGX_LAG]
                        _emit_gx(bb, tt, xts.pop(i - TGX_LAG))
                for i in range(len(steps) - TGX_LAG, len(steps)):
                    bb, tt = steps[i]
                    _emit_gx(bb, tt, xts.pop(i))

            # ---- stats AllGather (cheaper than AllReduce in the fabric:
            # no reduce pass) + local 8-way sum + all-batch GN prep ----
            def stats_ar():
                nc.gpsimd.dma_start(st_in[:], stats[:])
                nc.gpsimd.collective_compute(
                    "AllGather", ALU.bypass,
                    replica_groups=[list(range(N_CORES))],
                    ins=[st_in.opt()], outs=[st_gout.opt()],
                )
                stg8 = pers.tile([P, N_CORES, 2 * CO * B], F32)
                nc.gpsimd.dma_start(
                    stg8[:], st_gout.rearrange("g p a c b -> p g (a c b)"))
                nc.vector.reduce_sum(
                    stg[:].rearrange("p a c b -> p (a c b)"),
                    stg8[:].rearrange("p g a -> p a g"), AX)

            def prep_all(ps_pool):
                # group stats for all batches in one go: [2, CO, B]
                nf = 2 * CO * B
                pt1 = ps_pool.tile([P, C], F32, tag="yl", name="prep_ps")
                nc.tensor.matmul(
                    pt1[0:2, 0:nf], sel_sb[:],
                    stg[:].rearrange("p a b c -> p (a b c)"),
                    start=True, stop=True, skip_group_check=True)
                gst = small.tile([2, 2, CO, B], F32, tag="gst")
                nc.vector.tensor_copy(
                    gst[:].rearrange("p a b c -> p (a b c)"), pt1[0:2, 0:nf])
                mean_t = small.tile([2, CO, B], F32, tag="mean")
                nc.vector.tensor_scalar_mul(mean_t[:], gst[:, 0],
                                            1.0 / m_group)
                ex2_t = small.tile([2, CO, B], F32, tag="ex2")
                nc.vector.tensor_scalar_mul(ex2_t[:], gst[:, 1], 1.0 / m_group)
                var_t = small.tile([2, CO, B], F32, tag="var")
                nc.vector.tensor_tensor(var_t[:], mean_t[:], mean_t[:],
                                        ALU.mult)
                nc.vector.tensor_tensor(var_t[:], ex2_t[:], var_t[:],
                                        ALU.subtract)
                rstd_t = small.tile([2, CO, B], F32, tag="rstd")
                nc.scalar.activation(rstd_t[:], var_t[:], ACT.Sqrt,
                                     bias=eps_t[:])
                nc.vector.reciprocal(rstd_t[:], rstd_t[:])
                cg_t = small.tile([2, CO, B], F32, tag="cg")
                nc.vector.tensor_tensor(cg_t[:], mean_t[:], rstd_t[:],
                                        ALU.mult)
                nc.vector.tensor_scalar_mul(cg_t[:], cg_t[:], -1.0)
                rc2 = small.tile([2, 2, CO, B], F32, tag="rc2")
                nc.vector.tensor_copy(rc2[:, 0], rstd_t[:])
                nc.vector.tensor_copy(rc2[:, 1], cg_t[:])
                nc.tensor.matmul(
                    pt1[:, 512 - nf:512], selt_sb[:],
                    rc2[:].rearrange("p a b c -> p (a b c)"),
                    start=True, stop=True, skip_group_check=True)
                bc = small.tile([P, 2, CO, B], F32, tag="bc")
                nc.vector.tensor_copy(
                    bc[:].rearrange("p a b c -> p (a b c)"),
                    pt1[:, 512 - nf:512])
                # a = rstd*gnw, c = (-mean*rstd)*gnw + gnb, per batch
                for b in range(B):
                    nc.vector.tensor_tensor(a_sb[:, :, b], bc[:, 0, :, b],
                                            gnw_sb[:], ALU.mult)
                    nc.vector.tensor_tensor(c_sb[:, :, b], bc[:, 1, :, b],
                                            gnw_sb[:], ALU.mult)
                    nc.vector.tensor_tensor(c_sb[:, :, b], c_sb[:, :, b],
                                            gnb_sb[:], ALU.add)

            # ---- per-batch: correction vectors via PE ----
            def vec_prep(b, ps_pool):
                pv = small.tile([P, CO, 2], F32R, tag="pv")
                for co in range(CO):
                    nc.vector.tensor_tensor(pv[:, co, 0:1], a_sb[:, co, b:b+1],
                                            stg[:, 0, co, b:b + 1], ALU.mult)
                    nc.vector.tensor_copy(pv[:, co, 1:2], c_sb[:, co, b:b+1])
                # u and W@c rows accumulate in separate PSUM banks, both
                # at partition 0 (single-partition ops must sit at base 0)
                ub_c = small.tile([1, 2 * C], F32, tag="ubc")
                for half in range(2):
                    vpu = ps_pool.tile([P, C], F32, tag="yl",
                                       name=f"vecu{b}_{half}")
                    vpc = ps_pool.tile([P, C], F32, tag="yl",
                                       name=f"vecc{b}_{half}")
                    sl = slice(half * 512, half * 512 + 512)
                    for co in range(CO):
                        nc.tensor.matmul(
                            vpu[0:1, 0:C], pv[:, co, 0:1], wqk_sb[:, co, sl],
                            start=(co == 0), stop=(co == CO - 1),
                            skip_group_check=True)
                        nc.tensor.matmul(
                            vpc[0:1, 0:C], pv[:, co, 1:2], wqk_sb[:, co, sl],
                            start=(co == 0), stop=(co == CO - 1),
                            skip_group_check=True)
                    # u = A@Sx -> uqk8 (DVE scale-copy straight from PSUM)
                    nc.vector.tensor_scalar_mul(uqk8_l[b][:, sl],
                                                vpu[0:1, :], 1.0 / N_CORES)
                    nc.vector.tensor_copy(ub_c[:, sl], vpc[0:1, :])
                # bqk = W@c + qkv bias; nbq8 = N*bqk/8
                nc.vector.tensor_tensor(bqk_l[b][:], ub_c[:],
                                        qkb_sb[:], ALU.add)
                nc.vector.tensor_scalar_mul(nbq8_l[b][:], bqk_l[b][:],
                                            float(ntot) / N_CORES)
                if debug:
                    nc.sync.dma_start(dbg["dbg_vec"][:, b], ub[:])

            # ---- per-batch: Y = (a.Gx) Wk^T, Y' = a.Y, L = Wq^T Y' ----
            # both diag(a) GN scales fold into the evacuations: an in-place
            # DVE row-scale of the Gram and an ACT scale-copy of Y
            def y_l(b, gsb):
                for co in range(CO):
                    nc.vector.tensor_scalar_mul(gsb[:, co], gsb[:, co],
                                                a_sb[:, co, b:b + 1])
                y_sb = ysb.tile([P, CO, C], F32R, tag="y", name=f"ysb{b}")
                for c1 in range(CO):
                    yp = ylps.tile([P, C], F32, tag="yl", name=f"y{b}_{c1}")
                    for c2 in range(CO):
                        nc.tensor.matmul(
                            yp[:], gsb[:, c2, c1 * P:(c1 + 1) * P],
                            wqk_sb[:, c2, C:2 * C],
                            start=(c2 == 0), stop=(c2 == CO - 1))
                    nc.scalar.activation(y_sb[:, c1], yp[:], ACT.Identity,
                                         scale=a_sb[:, c1, b:b + 1])
                last_mm = None
                for dc in range(CO):
                    lp = ylps.tile([P, C], F32, tag="yl", name=f"l{b}_{dc}")
                    for c1 in range(CO):
                        nc.tensor.matmul(
                            lp[:], wqk_sb[:, c1, dc * P:(dc + 1) * P],
                            y_sb[:, c1], start=(c1 == 0), stop=False,
                            skip_group_check=True)
                    # rank-1 corrections on the two diagonal head blocks
                    for par in range(2):
                        hh = 2 * dc + par
                        rows = slice(par * 64, par * 64 + 64)
                        cols = slice(hh * 64, hh * 64 + 64)
                        tp = (0, 64) if par else None
                        ksl = slice(C + hh * 64, C + hh * 64 + 64)
                        qsl = slice(hh * 64, hh * 64 + 64)
                        nc.tensor.matmul(
                            lp[rows, cols], uqk8_l[b][:, qsl],
                            bqk_l[b][:, ksl], start=False, stop=False,
                            tile_position=tp, skip_group_check=True)
                        nc.tensor.matmul(
                            lp[rows, cols], bqk_l[b][:, qsl],
                            uqk8_l[b][:, ksl], start=False, stop=False,
                            tile_position=tp, skip_group_check=True)
                        last_mm = nc.tensor.matmul(
                            lp[rows, cols], nbq8_l[b][:, qsl],
                            bqk_l[b][:, ksl], start=False, stop=(par == 1),
                            tile_position=tp, skip_group_check=True)
                    # extract diagonal head blocks (ACT; DVE is loaded)
                    for par in range(2):
                        hh = 2 * dc + par
                        rows = slice(par * 64, par * 64 + 64)
                        nc.scalar.copy(
                            logits_sb[rows, b, dc, :],
                            lp[rows, hh * 64:hh * 64 + 64])
                return last_mm

            def lg_ar_in(p):
                # paired logits AllReduce: batches 2p, 2p+1 in one 256KB op
                nc.gpsimd.dma_start(lg_in_l[p][:],
                                    logits_sb[:, 2 * p:2 * p + 2])
                nc.gpsimd.collective_compute(
                    "AllReduce", ALU.add,
                    replica_groups=[list(range(N_CORES))],
                    ins=[lg_in_l[p].opt()], outs=[lg_out_l[p].opt()],
                )

            def lg_ar_out(p):
                # emitted after BOTH collectives so the copy-back of pair 0
                # does not block pair 1's input DMA in the Pool FIFO
                nc.gpsimd.dma_start(lg_full[:, 2 * p:2 * p + 2],
                                    lg_out_l[p][:])

            # ============ emit phase A/B in PE-schedule order ============
            # batches 2/3's stats come from dedicated gpsimd-queue loads so
            # the stats collective is not gated by the PE-paced transpose
            # feed (batches 0/1 stats read the transpose tiles directly)
            for b in range(B):
                load_t(b, stats_here=(b < 2))
            for b in (2, 3):
                for q in range(2):
                    cb = 2 * q
                    xs = xs3p.tile([P, 2, n_loc], F32, tag="xs3")
                    nc.gpsimd.dma_start(
                        xs[:], xin[b, cb * P:(cb + 2) * P, :]
                        .rearrange("(co ci) n -> ci co n", ci=P))
                    for cc in range(2):
                        for g in range(ngr):
                            nc.vector.bn_stats(
                                bst[:, b, cb + cc, g],
                                xs[:, cc, g * 512:(g + 1) * 512])
                stats_aggr(b)
            nc.gpsimd.dma_start(qkb_sb[:], qkb[:])
            nc.gpsimd.dma_start(vb_sb[:], vb2[:])
            nc.gpsimd.dma_start(ob_sb[:], ob2[:])
            nc.gpsimd.dma_start(gnw_sb[:], gnw2[:])
            nc.gpsimd.dma_start(gnb_sb[:], gnb2[:])
            stats_ar()
            wqk_sb = wqp.tile([P, CO, 2 * C], F32R)
            nc.gpsimd.dma_start(
                wqk_sb[:], wqk_t.rearrange("(co ci) o -> ci co o", ci=P))
            # V/out-proj weights: casting loads on the Pool queue, placed
            # before the logits collectives so they never block them
            wv_sb = wvp.tile([P, CO, C], F32R)
            wo_sb = wvp.tile([P, CO, C], F32R)
            nc.gpsimd.dma_start(
                wv_sb[:], wv_t.rearrange("(co ci) o -> ci co o", ci=P))
            nc.gpsimd.dma_start(
                wo_sb[:], wo_t.rearrange("(co ci) o -> ci co o", ci=P))

            t_gx_run([0, 1, 2])
            prep_all(ylps)
            vec_prep(0, ylps)
            vec_prep(1, ylps)
            y_l(0, gsb_l.pop(0))
            y_l(1, gsb_l.pop(1))
            lg_ar_in(0)
            vec_prep(2, ylps)
            vec_prep(3, ylps)
            t_gx_run([3])
            y_l(2, gsb_l.pop(2))
            y_l(3, gsb_l.pop(3))
            lg_ar_in(1)
            lg_ar_out(0)
            lg_ar_out(1)

            if debug:
                nc.sync.dma_start(dbg["dbg_a"][:], a_sb[:])
                nc.sync.dma_start(dbg["dbg_c"][:], c_sb[:])
                nc.sync.dma_start(dbg["dbg_stats"][:], stg[:])
                nc.sync.dma_start(dbg["dbg_logits"][:], lg_full[:])
            ab_stack.close()

            # ================= phase C =================
            with (
                tc.tile_pool(name="cpers", bufs=1) as cpers,
                tc.tile_pool(name="xc", bufs=5) as xc,
                tc.tile_pool(name="hp", bufs=2) as hp,
                tc.tile_pool(name="vp", bufs=4) as vp,
                tc.tile_pool(name="avp", bufs=2) as avp,
                tc.tile_pool(name="yp", bufs=2) as yp,
                tc.tile_pool(name="smp", bufs=4) as smp,
                tc.tile_pool(name="cps", bufs=6, space="PSUM") as cps,
                tc.tile_pool(name="trp", bufs=2, space="PSUM") as trp,
            ):
                attn_sb = cpers.tile([P, B, 4, D], F32)
                abd_f = cpers.tile([P, B * 4, P], F32)   # block-diag attn
                abd_r = cpers.tile([P, B * 4, P], F32R)  # transposed, f32r
                nc.vector.memset(abd_f[:], 0.0)

                def softmax_b(b):
                    for hp_i in range(4):
                        blk = lg_full[:, b, hp_i]
                        mx = smp.tile([P, 1], F32, tag="mx")
                        nc.vector.reduce_max(mx[:], blk, AX)
                        nbias = smp.tile([P, 1], F32, tag="nb")
                        nc.vector.tensor_scalar_mul(nbias[:], mx[:], -scale)
                        ex = attn_sb[:, b, hp_i]
                        nc.scalar.activation(ex, blk, ACT.Exp, bias=nbias[:],
                                             scale=scale)
                        sm = smp.tile([P, 1], F32, tag="sm")
                        nc.vector.reduce_sum(sm[:], ex, AX)
                        nc.vector.reciprocal(sm[:], sm[:])
                        nc.vector.tensor_scalar_mul(ex, ex, sm[:])
                        idx = b * 4 + hp_i
                        nc.vector.tensor_copy(abd_f[0:64, idx, 0:64],
                                              attn_sb[0:64, b, hp_i])
                        nc.vector.tensor_copy(abd_f[64:128, idx, 64:128],
                                              attn_sb[64:128, b, hp_i])

                def tr_attn(b):
                    for hp_i in range(4):
                        idx = b * 4 + hp_i
                        pt = trp.tile([P, P], F32, tag="pt")
                        nc.tensor.transpose(pt[:], abd_f[:, idx, :], ident[:])
                        nc.scalar.copy(abd_r[:, idx], pt[:])

                def emit_v(b, j):
                    xv = xin[b].rearrange("(co ci) n -> ci co n", ci=P)
                    xa = xc.tile([P, CO, NC], F32, tag="x")
                    eng = nc.sync if (j % 2 == 0) else nc.scalar
                    eng.dma_start(xa[:], xv[:, :, j * NC:(j + 1) * NC])
                    h = hp.tile([P, CO, NC], F32R, tag="h")
                    for co in range(CO):
                        nc.vector.tensor_scalar(
                            h[:, co], xa[:, co],
                            a_sb[:, co, b:b + 1], c_sb[:, co, b:b + 1],
                            ALU.mult, ALU.add)
                    v = vp.tile([P, CO, NC], F32R, tag="v")
                    for ot in range(CO):
                        ps_v = cps.tile([P, NC], F32, tag="c")
                        for co in range(CO):
                            nc.tensor.matmul(
                                ps_v[:], wv_sb[:, co, ot * P:(ot + 1) * P],
                                h[:, co], start=(co == 0), stop=(co == CO - 1))
                        nc.scalar.activation(v[:, ot], ps_v[:], ACT.Identity,
                                             bias=vb_sb[:, ot:ot + 1])
                    if debug and b == 0 and j == 0:
                        nc.gpsimd.dma_start(dbg["dbg_v"][:], v[:])
                    return xa, v

                def finish(b, j, xa, v):
                    av = avp.tile([P, CO, NC], F32R, tag="av")
                    for ot in range(CO):
                        ps_a = cps.tile([P, NC], F32, tag="c")
                        nc.tensor.matmul(ps_a[:], abd_r[:, b * 4 + ot],
                                         v[:, ot], start=True, stop=True)
                        nc.scalar.copy(av[:, ot], ps_a[:])
                    if debug and b == 0 and j == 0:
                        nc.gpsimd.dma_start(dbg["dbg_av"][:], av[:])
                    yv = yout[b].rearrange("(co ci) n -> ci co n", ci=P)
                    y_sb = yp.tile([P, CO, NC], F32, tag="y")
                    for ot in range(CO):
                        ps_o = cps.tile([P, NC], F32, tag="c")
                        for co in range(CO):
                            nc.tensor.matmul(
                                ps_o[:], wo_sb[:, co, ot * P:(ot + 1) * P],
                                av[:, co], start=(co == 0),
                                stop=(co == CO - 1))
                        # out bias (ACT, PSUM->SBUF) then +residual (DVE)
                        nc.scalar.activation(y_sb[:, ot], ps_o[:],
                                             ACT.Identity,
                                             bias=ob_sb[:, ot:ot + 1])
                        nc.vector.tensor_tensor(
                            y_sb[:, ot], y_sb[:, ot], xa[:, ot], ALU.add)
                    eng = nc.sync if (j % 2 == 0) else nc.scalar
                    eng.dma_start(yv[:, :, j * NC:(j + 1) * NC], y_sb[:])

                for b in range(B):
                    softmax_b(b)
                    pend = []
                    for j in range(nchunks):
                        pend.append((j, *emit_v(b, j)))
                    tr_attn(b)
                    for j, xa, v in pend:
                        finish(b, j, xa, v)
                if debug:
                    nc.sync.dma_start(dbg["dbg_attn"][:], attn_sb[:])

    return nc


_WAITSPLIT_COUNTER = [0]


def _split_waits(nc, limit: int = 1):
    """Walrus in this container rejects instructions with more than one sync
    wait; split extras onto injected NoOps on the same engine."""
    n_split = 0
    for fn in nc.m.functions:
        for bb in fn.blocks:
            insts = list(bb.instructions)
            out = []
            changed = False
            for inst in insts:
                si = inst.sync_info
                waits = list(si.on_wait) if si is not None and si.on_wait \
                    else []
                if len(waits) > limit:
                    keep = waits[-limit:]
                    extra = waits[:-limit]
                    for i in range(0, len(extra), limit):
                        chunk = extra[i:i + limit]
                        _WAITSPLIT_COUNTER[0] += 1
                        nop = mybir.InstNoOp(
                            name=f"waitsplit-{_WAITSPLIT_COUNTER[0]}",
                            ins=[], outs=[])
                        nop.engine = inst.engine
                        nop.sync_info = mybir.SyncInfo(
                            on_wait=chunk, on_update=[])
                        out.append(nop)
                    si.on_wait = keep
                    n_split += 1
                    changed = True
                out.append(inst)
            if changed:
                bb.instructions = out
    return n_split


_CACHE = {}


def _get_module(n_loc, split=True, debug=False):
    key = (n_loc, split, debug)
    if key not in _CACHE:
        nc = build_module(n_loc, debug=debug)
        if split:
            _split_waits(nc, limit=1)
        _CACHE[key] = nc
    return _CACHE[key]


def make_in_maps(inputs, n_loc=None):
    x = np.ascontiguousarray(np.asarray(inputs["x"], dtype=np.float32))
    qkv_w = np.asarray(inputs["qkv_w"], dtype=np.float32)
    qkv_b = np.asarray(inputs["qkv_b"], dtype=np.float32)
    out_w = np.asarray(inputs["out_w"], dtype=np.float32)
    out_b = np.asarray(inputs["out_b"], dtype=np.float32)
    gn_w = np.asarray(inputs["gn_weight"], dtype=np.float32)
    gn_b = np.asarray(inputs["gn_bias"], dtype=np.float32)

    n_tot = int(np.prod(x.shape[2:]))
    if n_loc is None:
        n_loc = n_tot // N_CORES
    xf = x.reshape(B, C, n_tot)

    wqk_t = np.ascontiguousarray(_round_tf32(qkv_w[0:2 * C].T))
    wv_t = np.ascontiguousarray(_round_tf32(qkv_w[2 * C:3 * C].T))
    wo_t = np.ascontiguousarray(_round_tf32(out_w.T))
    qkb = np.ascontiguousarray(qkv_b[0:2 * C].reshape(1, 2 * C))
    vb2 = np.ascontiguousarray(qkv_b[2 * C:3 * C].reshape(CO, P).T)
    ob2 = np.ascontiguousarray(out_b.reshape(CO, P).T)
    gnw2 = np.ascontiguousarray(gn_w.reshape(CO, P).T)
    gnb2 = np.ascontiguousarray(gn_b.reshape(CO, P).T)

    shared = dict(wqk_t=wqk_t, wv_t=wv_t, wo_t=wo_t, qkb=qkb, vb2=vb2,
                  ob2=ob2, gnw2=gnw2, gnb2=gnb2)
    in_maps = []
    for c in range(N_CORES):
        sl = np.ascontiguousarray(xf[:, :, c * n_loc:(c + 1) * n_loc])
        in_maps.append({"xin": sl, **shared})
    return in_maps


def run(inputs, n_loc=None, debug=False, **kw):
    x = np.asarray(inputs["x"])
    n_tot = int(np.prod(x.shape[2:]))
    if n_loc is None:
        n_loc = n_tot // N_CORES
    nc = _get_module(n_loc, debug=debug)
    in_maps = make_in_maps(inputs, n_loc)
    res = bass_utils.run_bass_kernel_spmd(
        nc, in_maps, core_ids=list(range(N_CORES)), **kw)
    y = np.concatenate([res.results[c]["yout"] for c in range(N_CORES)],
                       axis=2)
    return y, res


def kernel(**inputs) -> np.ndarray:
    x = np.asarray(inputs["x"])
    y, _ = run(inputs)
    return y.reshape(x.shape).astype(np.asarray(x).dtype)


# revision 96
# speedup vs baseline: 1.1241x; 1.1241x over previous
"""AttentionBlock3D on 8 Trainium2 NeuronCores — Gram-matrix restructure.

Math (see reference.py): GroupNorm(8 groups) -> qkv 1x1 conv -> channel
attention (contract over tokens N, softmax over last d=64) -> out proj ->
residual.

Sharding: N = T*H*W = 16384 tokens split 8 ways (2048/core); every core
holds all 4 batches of its token slice.  Cross-core state: one GroupNorm
stats AllReduce (16 KB, all batches merged) + one logits AllReduce per
batch (128 KB each).

Key identity: the channel-attention logits contract over tokens, so
  L = Q K^T = Aq Gx Ak^T + uq bk~^T + bq~ uk^T + N bq~ bk~^T
with Gx = sum_n x x^T (Gram of RAW x — independent of GroupNorm stats!),
Aq = Wq diag(a), uq = Aq Sx, bq~ = Wq c + bq (a, c = GN scale/shift).
Gx is computed on the PE from t~4us (PE-transposed x tiles), entirely
overlapping the stats pipeline + AllReduce; the remaining per-batch work
(Y = Gx Ak^T, L = Aq Y, rank-1 corrections) is tiny.  Per-core PE work
drops from ~300us (baseline QK-projection path) to ~240us and there is
no startup bubble.

Phases (per-core):
  A: x streamed once ([128 x 2048] tiles, 4 DMA queues); bn_stats (DVE)
     + Identity/Square accum (ACT) -> per-channel sum/sumsq; ONE stats
     AllReduce for all batches; PE meanwhile transposes x blocks and
     accumulates Gx(b) per batch in PSUM.
  B: per batch: aqkT = diag(a)*[Wq|Wk]^T (DVE row-scale), Y = Gx aqkT_k,
     L = aqkT_q^T Y (+ rank-1 bias/GN corrections in fp32), extract the 8
     diagonal 64x64 head blocks -> per-batch logits AllReduce.
  C: per batch: softmax + PE blockdiag-transpose, then per 512-token
     chunk: h = a*x+c (DVE), V proj, attn@v, out proj, +residual, store.

All big matmuls run in float32r (free dim 512 -> 1 cycle/row); the rank-1
correction matmuls and transposes are exact fp32.
"""

import numpy as np

import concourse.bass as bass
import concourse.mybir as mybir
import concourse.tile as tile
from concourse import bass_utils

F32 = mybir.dt.float32
F32R = mybir.dt.float32r
AX = mybir.AxisListType.X
ALU = mybir.AluOpType
ACT = mybir.ActivationFunctionType

N_CORES = 8
B, C, T, H, W = 4, 512, 16, 32, 32
N_TOT = T * H * W            # 16384
NH, D = 8, 64                # heads, head dim
G = 8                        # groupnorm groups
EPS = 1e-5
P = 128
CO = C // P                  # 4 channel chunks
NC = 512                     # phase-C token chunk size


def _round_tf32(a: np.ndarray) -> np.ndarray:
    """Round fp32 to fp32r (keep 10 explicit mantissa bits, RNE)."""
    u = a.astype(np.float32).view(np.uint32).astype(np.uint64)
    u = (u + 0x1000 + ((u >> 13) & 1)) & 0xFFFFE000
    return u.astype(np.uint32).view(np.float32)


def build_module(n_loc: int, debug: bool = False):
    NT = n_loc // P              # token blocks per batch (16)
    nchunks = n_loc // NC        # phase-C chunks per batch (4)
    ngr = n_loc // 512           # bn_stats groups per tile
    ntot = n_loc * N_CORES
    m_group = (C // G) * ntot    # elements per (b, group) stat
    scale = float(D) ** -0.5

    nc = bass.Bass("TRN2", target_bir_lowering=False, debug=False,
                   num_devices=N_CORES)

    xin = nc.dram_tensor("xin", [B, C, n_loc], F32, kind="ExternalInput").ap()
    wqk_t = nc.dram_tensor("wqk_t", [C, 2 * C], F32, kind="ExternalInput").ap()
    wv_t = nc.dram_tensor("wv_t", [C, C], F32, kind="ExternalInput").ap()
    wo_t = nc.dram_tensor("wo_t", [C, C], F32, kind="ExternalInput").ap()
    qkb = nc.dram_tensor("qkb", [1, 2 * C], F32, kind="ExternalInput").ap()
    vb2 = nc.dram_tensor("vb2", [P, CO], F32, kind="ExternalInput").ap()
    ob2 = nc.dram_tensor("ob2", [P, CO], F32, kind="ExternalInput").ap()
    gnw2 = nc.dram_tensor("gnw2", [P, CO], F32, kind="ExternalInput").ap()
    gnb2 = nc.dram_tensor("gnb2", [P, CO], F32, kind="ExternalInput").ap()
    yout = nc.dram_tensor("yout", [B, C, n_loc], F32, kind="ExternalOutput").ap()
    dbg = {}
    if debug:
        for nm, shp in [("dbg_stats", [P, 2, CO, B]), ("dbg_a", [P, CO, B]),
                        ("dbg_c", [P, CO, B]), ("dbg_gx", [P, CO, C]),
                        ("dbg_logits", [P, B, 4, D]),
                        ("dbg_attn", [P, B, 4, D]),
                        ("dbg_vec", [2, B, 2 * C]),
                        ("dbg_v", [P, CO, NC]), ("dbg_av", [P, CO, NC])]:
            dbg[nm] = nc.dram_tensor(nm, shp, F32, kind="ExternalOutput").ap()

    from concourse.masks import make_identity
    from bass_rust import add_dep_helper as _adh

    with tile.TileContext(nc) as tc:
        with (
            tc.tile_pool(name="persist", bufs=1) as pers,
            tc.tile_pool(name="wvpool", bufs=1) as wvp,
            tc.tile_pool(name="dram", bufs=1, space="DRAM") as dram,
        ):
            # ------------- persistent tiles (consts on gpsimd queue so
            # the HWDGE queues start streaming x at t=0) -------------
            qkb_sb = pers.tile([1, 2 * C], F32)
            vb_sb = pers.tile([P, CO], F32)
            ob_sb = pers.tile([P, CO], F32)
            gnw_sb = pers.tile([P, CO], F32)
            gnb_sb = pers.tile([P, CO], F32)

            ident = pers.tile([P, P], F32)
            make_identity(nc, ident[:])
            sel_sb = pers.tile([P, 2], F32)
            nc.vector.memset(sel_sb[:], 0.0)
            nc.vector.memset(sel_sb[0:64, 0:1], 1.0)
            nc.vector.memset(sel_sb[64:128, 1:2], 1.0)
            selt_sb = pers.tile([2, P], F32)
            with tc.tile_pool(name="selps", bufs=1, space="PSUM") as selps:
                sel_pt = selps.tile([2, P], F32)
                nc.tensor.transpose(sel_pt[:], sel_sb[:], ident[:])
                nc.vector.tensor_copy(selt_sb[:], sel_pt[:])

            a_sb = pers.tile([P, CO, B], F32)     # GN scale per (ci,co,b)
            c_sb = pers.tile([P, CO, B], F32)     # GN shift
            logits_sb = pers.tile([P, B, 4, D], F32)
            lg_full = pers.tile([P, B, 4, D], F32)

            stats = pers.tile([P, 2, CO, B], F32)   # local sum/sumsq
            stg = pers.tile([P, 2, CO, B], F32)     # global (post-AR)
            bst = pers.tile([P, B, CO, ngr, 6], F32)
            st_in = dram.tile([P, 2, CO, B], F32, name="st_in")
            st_gout = dram.tile([N_CORES, P, 2, CO, B], F32, name="st_gout")
            lg_in_l = [dram.tile([P, 2, 4, D], F32, name=f"lg_in{pp}")
                       for pp in range(B // 2)]
            lg_out_l = [dram.tile([P, 2, 4, D], F32, name=f"lg_out{pp}")
                        for pp in range(B // 2)]

            # rank-1 correction vectors (uqk/8, bqk, N*bqk/8) per batch, in
            # bf16 (the corrections are small relative to the logits, and
            # bf16 halves partition-0 SBUF pressure + runs 1 cycle/row)
            BF16 = mybir.dt.bfloat16
            cvec = pers.tile([1, 3, B, 2 * C], BF16)
            uqk8_l = [cvec[0:1, 0, bb] for bb in range(B)]
            bqk_l = [cvec[0:1, 1, bb] for bb in range(B)]
            nbq8_l = [cvec[0:1, 2, bb] for bb in range(B)]

            eps_t = pers.tile([2, 1], F32)
            nc.vector.memset(eps_t[:], EPS)

            # ================= phase A/B =================
            import contextlib
            ab_stack = contextlib.ExitStack()
            xp = ab_stack.enter_context(tc.tile_pool(name="xp", bufs=4))
            xs3p = ab_stack.enter_context(tc.tile_pool(name="xs3p", bufs=2))
            xtp = ab_stack.enter_context(tc.tile_pool(name="xtp", bufs=3))
            wqp = ab_stack.enter_context(tc.tile_pool(name="wqp", bufs=1))
            gxsb = ab_stack.enter_context(tc.tile_pool(name="gxsb", bufs=3))
            ysb = ab_stack.enter_context(tc.tile_pool(name="ysb", bufs=1))
            small = ab_stack.enter_context(tc.tile_pool(name="small", bufs=1))
            tps = ab_stack.enter_context(
                tc.tile_pool(name="tps", bufs=2, space="PSUM"))
            gxps = ab_stack.enter_context(
                tc.tile_pool(name="gxps", bufs=1, space="PSUM"))
            ylps = ab_stack.enter_context(
                tc.tile_pool(name="ylps", bufs=2, space="PSUM"))

            # x is loaded twice in phase A, as [P, 2, *] co-PAIR tiles (one
            # DMA per pair — per-DMA overhead is ~2.2us so bigger is better)
            # on the two HWDGE queues (SP: co 0/1, ACT: co 2/3); the gpsimd
            # queue stays free so the stats collective fires immediately:
            #  - transpose feed: [P, 2, 1024] halves (PE-paced)
            #  - stats feed: [P, 2, 2048], consumed right away (DVE
            #    bn_stats for co 0/1, ACT accum passes for co 2/3)
            qpair = [(nc.sync, 0), (nc.scalar, 2)]
            NJ = 2                      # transpose halves per (b, pair)
            gh = ngr // NJ              # bn_stats windows per half per co
            xa_t = {}

            def load_t(b, stats_here=True):
                # single x pass: [P, 2, 1024] co-pair halves feed BOTH the
                # PE transposes and (for b<3) the DVE bn_stats
                w = n_loc // NJ
                for jj in range(NJ):
                    for q, (eng, cb) in enumerate(qpair):
                        xa = xp.tile([P, 2, w], F32, tag=f"xt{q}",
                                     name=f"xt{b}_{q}_{jj}")
                        eng.dma_start(
                            xa[:],
                            xin[b, cb * P:(cb + 2) * P, jj * w:(jj + 1) * w]
                            .rearrange("(co ci) n -> ci co n", ci=P))
                        xa_t[(b, q, jj)] = xa
                        if stats_here:
                            for cc in range(2):
                                for g in range(gh):
                                    nc.vector.bn_stats(
                                        bst[:, b, cb + cc, jj * gh + g],
                                        xa[:, cc, g * 512:(g + 1) * 512])
                if stats_here:
                    stats_aggr(b)

            def stats_aggr(b):
                for co in range(CO):
                    mvt = small.tile([P, 2], F32, tag="mvt")
                    nc.vector.bn_aggr(mvt[:], bst[:, b, co])
                    nc.vector.tensor_scalar_mul(
                        stats[:, 0, co, b:b + 1], mvt[:, 0:1], float(n_loc))
                    nc.vector.tensor_tensor(
                        stats[:, 1, co, b:b + 1], mvt[:, 0:1],
                        mvt[:, 0:1], ALU.mult)
                    nc.vector.tensor_tensor(
                        stats[:, 1, co, b:b + 1],
                        stats[:, 1, co, b:b + 1], mvt[:, 1:2],
                        ALU.add)
                    nc.vector.tensor_scalar_mul(
                        stats[:, 1, co, b:b + 1],
                        stats[:, 1, co, b:b + 1], float(n_loc))

            # ---- PE: transpose x + accumulate Gram, software-pipelined
            # with lag so the ACT evacuation never stalls the PE ----
            TGX_LAG = 2
            gx_ps = {}
            gsb_l = {}

            def _emit_t(b, tb):
                tpb = NT // NJ          # tok-blocks per transpose half
                pt = tps.tile([P, C], F32, tag="t")
                for co in range(CO):
                    q, cc = divmod(co, 2)
                    nc.tensor.transpose(
                        pt[:, co * P:(co + 1) * P],
                        xa_t[(b, q, tb // tpb)][:, cc,
                                                (tb % tpb) * P:
                                                (tb % tpb + 1) * P],
                        ident[:])
                xt = xtp.tile([P, C], F32R, tag="xt")
                nc.scalar.copy(xt[:], pt[:])
                return xt

            def _emit_gx(b, tb, xt):
                if tb == 0:
                    gx_ps[b] = [gxps.tile([P, C], F32, tag=f"gx{co}",
                                          name=f"gx{b}_{co}")
                                for co in range(CO)]
                for co in range(CO):
                    nc.tensor.matmul(
                        gx_ps[b][co][:], xt[:, co * P:(co + 1) * P],
                        xt[:], start=(tb == 0), stop=(tb == NT - 1))
                if tb == NT - 1:
                    # evacuate Gram to SBUF (ACT — DVE runs bn_stats and
                    # must not serialize the Gram pipeline behind them)
                    gsb = gxsb.tile([P, CO, C], F32R, tag="gx",
                                    name=f"gxsb{b}")
                    for co in range(CO):
                        nc.scalar.copy(gsb[:, co], gx_ps[b][co][:])
                    if debug and b == 0:
                        nc.gpsimd.dma_start(dbg["dbg_gx"][:], gsb[:])
                    gsb_l[b] = gsb

            def t_gx_run(batches):
                steps = [(b, tb) for b in batches for tb in range(NT)]
                xts = {}
                for i, (b, tb) in enumerate(steps):
                    xts[i] = _emit_t(b, tb)
                    if i >= TGX_LAG:
                        bb, tt = steps[i - TGX_LAG]
                        _emit_gx(bb, tt, xts.pop(i - TGX_LAG))
                for i in range(len(steps) - TGX_LAG, len(steps)):
                    bb, tt = steps[i]
                    _emit_gx(bb, tt, xts.pop(i))

            # ---- stats AllGather (cheaper than AllReduce in the fabric:
            # no reduce pass) + local 8-way sum + all-batch GN prep ----
            def stats_ar():
                nc.gpsimd.dma_start(st_in[:], stats[:])
                nc.gpsimd.collective_compute(
                    "AllGather", ALU.bypass,
                    replica_groups=[list(range(N_CORES))],
                    ins=[st_in.opt()], outs=[st_gout.opt()],
                )
                stg8 = pers.tile([P, N_CORES, 2 * CO * B], F32)
                nc.gpsimd.dma_start(
                    stg8[:], st_gout.rearrange("g p a c b -> p g (a c b)"))
                nc.vector.reduce_sum(
                    stg[:].rearrange("p a c b -> p (a c b)"),
                    stg8[:].rearrange("p g a -> p a g"), AX)

            def prep_all(ps_pool):
                # group stats for all batches in one go: [2, CO, B]
                nf = 2 * CO * B
                pt1 = ps_pool.tile([P, C], F32, tag="yl", name="prep_ps")
                nc.tensor.matmul(
                    pt1[0:2, 0:nf], sel_sb[:],
                    stg[:].rearrange("p a b c -> p (a b c)"),
                    start=True, stop=True, skip_group_check=True)
                gst = small.tile([2, 2, CO, B], F32, tag="gst")
                nc.vector.tensor_copy(
                    gst[:].rearrange("p a b c -> p (a b c)"), pt1[0:2, 0:nf])
                mean_t = small.tile([2, CO, B], F32, tag="mean")
                nc.vector.tensor_scalar_mul(mean_t[:], gst[:, 0],
                                            1.0 / m_group)
                ex2_t = small.tile([2, CO, B], F32, tag="ex2")
                nc.vector.tensor_scalar_mul(ex2_t[:], gst[:, 1], 1.0 / m_group)
                var_t = small.tile([2, CO, B], F32, tag="var")
                nc.vector.tensor_tensor(var_t[:], mean_t[:], mean_t[:],
                                        ALU.mult)
                nc.vector.tensor_tensor(var_t[:], ex2_t[:], var_t[:],
                                        ALU.subtract)
                rstd_t = small.tile([2, CO, B], F32, tag="rstd")
                nc.scalar.activation(rstd_t[:], var_t[:], ACT.Sqrt,
                                     bias=eps_t[:])
                nc.vector.reciprocal(rstd_t[:], rstd_t[:])
                cg_t = small.tile([2, CO, B], F32, tag="cg")
                nc.vector.tensor_tensor(cg_t[:], mean_t[:], rstd_t[:],
                                        ALU.mult)
                nc.vector.tensor_scalar_mul(cg_t[:], cg_t[:], -1.0)
                rc2 = small.tile([2, 2, CO, B], F32, tag="rc2")
                nc.vector.tensor_copy(rc2[:, 0], rstd_t[:])
                nc.vector.tensor_copy(rc2[:, 1], cg_t[:])
                nc.tensor.matmul(
                    pt1[:, 512 - nf:512], selt_sb[:],
                    rc2[:].rearrange("p a b c -> p (a b c)"),
                    start=True, stop=True, skip_group_check=True)
                bc = small.tile([P, 2, CO, B], F32, tag="bc")
                nc.vector.tensor_copy(
                    bc[:].rearrange("p a b c -> p (a b c)"),
                    pt1[:, 512 - nf:512])
                # a = rstd*gnw, c = (-mean*rstd)*gnw + gnb, per batch
                for b in range(B):
                    nc.vector.tensor_tensor(a_sb[:, :, b], bc[:, 0, :, b],
                                            gnw_sb[:], ALU.mult)
                    nc.vector.tensor_tensor(c_sb[:, :, b], bc[:, 1, :, b],
                                            gnw_sb[:], ALU.mult)
                    nc.vector.tensor_tensor(c_sb[:, :, b], c_sb[:, :, b],
                                            gnb_sb[:], ALU.add)

            # ---- per-batch: correction vectors via PE ----
            def vec_prep(b, ps_pool):
                pv = small.tile([P, CO, 2], F32R, tag="pv")
                for co in range(CO):
                    nc.vector.tensor_tensor(pv[:, co, 0:1], a_sb[:, co, b:b+1],
                                            stg[:, 0, co, b:b + 1], ALU.mult)
                    nc.vector.tensor_copy(pv[:, co, 1:2], c_sb[:, co, b:b+1])
                # u and W@c rows accumulate in separate PSUM banks, both
                # at partition 0 (single-partition ops must sit at base 0)
                ub_c = small.tile([1, 2 * C], F32, tag="ubc")
                for half in range(2):
                    vpu = ps_pool.tile([P, C], F32, tag="yl",
                                       name=f"vecu{b}_{half}")
                    vpc = ps_pool.tile([P, C], F32, tag="yl",
                                       name=f"vecc{b}_{half}")
                    sl = slice(half * 512, half * 512 + 512)
                    for co in range(CO):
                        nc.tensor.matmul(
                            vpu[0:1, 0:C], pv[:, co, 0:1], wqk_sb[:, co, sl],
                            start=(co == 0), stop=(co == CO - 1),
                            skip_group_check=True)
                        nc.tensor.matmul(
                            vpc[0:1, 0:C], pv[:, co, 1:2], wqk_sb[:, co, sl],
                            start=(co == 0), stop=(co == CO - 1),
                            skip_group_check=True)
                    # u = A@Sx -> uqk8 (DVE scale-copy straight from PSUM)
                    nc.vector.tensor_scalar_mul(uqk8_l[b][:, sl],
                                                vpu[0:1, :], 1.0 / N_CORES)
                    nc.vector.tensor_copy(ub_c[:, sl], vpc[0:1, :])
                # bqk = W@c + qkv bias; nbq8 = N*bqk/8
                nc.vector.tensor_tensor(bqk_l[b][:], ub_c[:],
                                        qkb_sb[:], ALU.add)
                nc.vector.tensor_scalar_mul(nbq8_l[b][:], bqk_l[b][:],
                                            float(ntot) / N_CORES)
                if debug:
                    nc.sync.dma_start(dbg["dbg_vec"][:, b], ub[:])

            # ---- per-batch: Y = (a.Gx) Wk^T, Y' = a.Y, L = Wq^T Y' ----
            # both diag(a) GN scales fold into the evacuations: an in-place
            # DVE row-scale of the Gram and an ACT scale-copy of Y
            def y_l(b, gsb):
                for co in range(CO):
                    nc.vector.tensor_scalar_mul(gsb[:, co], gsb[:, co],
                                                a_sb[:, co, b:b + 1])
                y_sb = ysb.tile([P, CO, C], F32R, tag="y", name=f"ysb{b}")
                for c1 in range(CO):
                    yp = ylps.tile([P, C], F32, tag="yl", name=f"y{b}_{c1}")
                    for c2 in range(CO):
                        nc.tensor.matmul(
                            yp[:], gsb[:, c2, c1 * P:(c1 + 1) * P],
                            wqk_sb[:, c2, C:2 * C],
                            start=(c2 == 0), stop=(c2 == CO - 1))
                    nc.scalar.activation(y_sb[:, c1], yp[:], ACT.Identity,
                                         scale=a_sb[:, c1, b:b + 1])
                last_mm = None
                for dc in range(CO):
                    lp = ylps.tile([P, C], F32, tag="yl", name=f"l{b}_{dc}")
                    for c1 in range(CO):
                        nc.tensor.matmul(
                            lp[:], wqk_sb[:, c1, dc * P:(dc + 1) * P],
                            y_sb[:, c1], start=(c1 == 0), stop=False,
                            skip_group_check=True)
                    # rank-1 corrections on the two diagonal head blocks
                    for par in range(2):
                        hh = 2 * dc + par
                        rows = slice(par * 64, par * 64 + 64)
                        cols = slice(hh * 64, hh * 64 + 64)
                        tp = (0, 64) if par else None
                        ksl = slice(C + hh * 64, C + hh * 64 + 64)
                        qsl = slice(hh * 64, hh * 64 + 64)
                        nc.tensor.matmul(
                            lp[rows, cols], uqk8_l[b][:, qsl],
                            bqk_l[b][:, ksl], start=False, stop=False,
                            tile_position=tp, skip_group_check=True)
                        nc.tensor.matmul(
                            lp[rows, cols], bqk_l[b][:, qsl],
                            uqk8_l[b][:, ksl], start=False, stop=False,
                            tile_position=tp, skip_group_check=True)
                        last_mm = nc.tensor.matmul(
                            lp[rows, cols], nbq8_l[b][:, qsl],
                            bqk_l[b][:, ksl], start=False, stop=(par == 1),
                            tile_position=tp, skip_group_check=True)
                    # extract diagonal head blocks (ACT; DVE is loaded)
                    for par in range(2):
                        hh = 2 * dc + par
                        rows = slice(par * 64, par * 64 + 64)
                        nc.scalar.copy(
                            logits_sb[rows, b, dc, :],
                            lp[rows, hh * 64:hh * 64 + 64])
                return last_mm

            def lg_ar_in(p):
                # paired logits AllReduce: batches 2p, 2p+1 in one 256KB op
                nc.gpsimd.dma_start(lg_in_l[p][:],
                                    logits_sb[:, 2 * p:2 * p + 2])
                nc.gpsimd.collective_compute(
                    "AllReduce", ALU.add,
                    replica_groups=[list(range(N_CORES))],
                    ins=[lg_in_l[p].opt()], outs=[lg_out_l[p].opt()],
                )

            def lg_ar_out(p):
                # emitted after BOTH collectives so the copy-back of pair 0
                # does not block pair 1's input DMA in the Pool FIFO
                nc.gpsimd.dma_start(lg_full[:, 2 * p:2 * p + 2],
                                    lg_out_l[p][:])

            # ============ emit phase A/B in PE-schedule order ============
            # stats feed: batches 0/1/2 read the transpose tiles (b2's
            # bn_stats emitted LAST — its tiles arrive at PE pace); batch
            # 3 loads on the (otherwise idle) gpsimd queue early
            load_t(0, stats_here=True)
            load_t(1, stats_here=True)
            load_t(2, stats_here=False)
            load_t(3, stats_here=False)
            wh = n_loc // 2
            for jj in range(2):
                for q in range(2):
                    cb = 2 * q
                    xs = xs3p.tile([P, 2, wh], F32, tag="xs3")
                    nc.gpsimd.dma_start(
                        xs[:], xin[3, cb * P:(cb + 2) * P,
                                   jj * wh:(jj + 1) * wh]
                        .rearrange("(co ci) n -> ci co n", ci=P))
                    for cc in range(2):
                        for g in range(gh):
                            nc.vector.bn_stats(
                                bst[:, 3, cb + cc, jj * gh + g],
                                xs[:, cc, g * 512:(g + 1) * 512])
            stats_aggr(3)
            for jj in range(NJ):
                for q, (eng, cb) in enumerate(qpair):
                    for cc in range(2):
                        for g in range(gh):
                            nc.vector.bn_stats(
                                bst[:, 2, cb + cc, jj * gh + g],
                                xa_t[(2, q, jj)][:, cc,
                                                 g * 512:(g + 1) * 512])
            stats_aggr(2)
            nc.gpsimd.dma_start(qkb_sb[:], qkb[:])
            nc.gpsimd.dma_start(vb_sb[:], vb2[:])
            nc.gpsimd.dma_start(ob_sb[:], ob2[:])
            nc.gpsimd.dma_start(gnw_sb[:], gnw2[:])
            nc.gpsimd.dma_start(gnb_sb[:], gnb2[:])
            stats_ar()
            wqk_sb = wqp.tile([P, CO, 2 * C], F32R)
            nc.gpsimd.dma_start(
                wqk_sb[:], wqk_t.rearrange("(co ci) o -> ci co o", ci=P))
            # V/out-proj weights: casting loads on the Pool queue, placed
            # before the logits collectives so they never block them
            wv_sb = wvp.tile([P, CO, C], F32R)
            wo_sb = wvp.tile([P, CO, C], F32R)
            nc.gpsimd.dma_start(
                wv_sb[:], wv_t.rearrange("(co ci) o -> ci co o", ci=P))
            nc.gpsimd.dma_start(
                wo_sb[:], wo_t.rearrange("(co ci) o -> ci co o", ci=P))

            t_gx_run([0, 1, 2])
            prep_all(ylps)
            vec_prep(0, ylps)
            vec_prep(1, ylps)
            y_l(0, gsb_l.pop(0))
            y_l(1, gsb_l.pop(1))
            lg_ar_in(0)
            vec_prep(2, ylps)
            vec_prep(3, ylps)
            t_gx_run([3])
            y_l(2, gsb_l.pop(2))
            y_l(3, gsb_l.pop(3))
            lg_ar_in(1)   # pair-1 input DMA precedes pair-0's copy-back in
            lg_ar_out(0)  # the Pool FIFO so AR23 starts the moment AR01 ends
            lg_ar_out(1)
            _ = gh  # silence lint; gh used by load_t

            if debug:
                nc.sync.dma_start(dbg["dbg_a"][:], a_sb[:])
                nc.sync.dma_start(dbg["dbg_c"][:], c_sb[:])
                nc.sync.dma_start(dbg["dbg_stats"][:], stg[:])
                nc.sync.dma_start(dbg["dbg_logits"][:], lg_full[:])
            ab_stack.close()

            # ================= phase C =================
            with (
                tc.tile_pool(name="cpers", bufs=1) as cpers,
                tc.tile_pool(name="xc", bufs=5) as xc,
                tc.tile_pool(name="hp", bufs=2) as hp,
                tc.tile_pool(name="vp", bufs=4) as vp,
                tc.tile_pool(name="avp", bufs=2) as avp,
                tc.tile_pool(name="yp", bufs=2) as yp,
                tc.tile_pool(name="smp", bufs=4) as smp,
                tc.tile_pool(name="cps", bufs=6, space="PSUM") as cps,
                tc.tile_pool(name="trp", bufs=2, space="PSUM") as trp,
            ):
                attn_sb = cpers.tile([P, B, 4, D], F32)
                abd_f = cpers.tile([P, B * 4, P], F32)   # block-diag attn
                abd_r = cpers.tile([P, B * 4, P], F32R)  # transposed, f32r
                nc.vector.memset(abd_f[:], 0.0)
                # transposed out-bias rows + ones row: the out bias is
                # added by a rank-1 PE matmul folded into the out-proj
                obt4 = cpers.tile([1, CO, P], mybir.dt.bfloat16)
                ones4 = cpers.tile([1, NC], mybir.dt.bfloat16)
                nc.vector.memset(ones4[:], 1.0)
                for ot in range(CO):
                    pt = trp.tile([P, P], F32, tag="pt", name=f"ob{ot}")
                    nc.tensor.transpose(pt[0:1, :], ob_sb[:, ot:ot + 1],
                                        ident[:])
                    nc.scalar.copy(obt4[:, ot], pt[0:1, :])

                def softmax_b(b):
                    for hp_i in range(4):
                        blk = lg_full[:, b, hp_i]
                        mx = smp.tile([P, 1], F32, tag="mx")
                        nc.vector.reduce_max(mx[:], blk, AX)
                        nbias = smp.tile([P, 1], F32, tag="nb")
                        nc.vector.tensor_scalar_mul(nbias[:], mx[:], -scale)
                        ex = attn_sb[:, b, hp_i]
                        nc.scalar.activation(ex, blk, ACT.Exp, bias=nbias[:],
                                             scale=scale)
                        sm = smp.tile([P, 1], F32, tag="sm")
                        nc.vector.reduce_sum(sm[:], ex, AX)
                        nc.vector.reciprocal(sm[:], sm[:])
                        nc.vector.tensor_scalar_mul(ex, ex, sm[:])
                        idx = b * 4 + hp_i
                        nc.vector.tensor_copy(abd_f[0:64, idx, 0:64],
                                              attn_sb[0:64, b, hp_i])
                        nc.vector.tensor_copy(abd_f[64:128, idx, 64:128],
                                              attn_sb[64:128, b, hp_i])

                def tr_attn(b):
                    for hp_i in range(4):
                        idx = b * 4 + hp_i
                        pt = trp.tile([P, P], F32, tag="pt")
                        nc.tensor.transpose(pt[:], abd_f[:, idx, :], ident[:])
                        nc.scalar.copy(abd_r[:, idx], pt[:])

                def emit_v(b, j):
                    xv = xin[b].rearrange("(co ci) n -> ci co n", ci=P)
                    xa = xc.tile([P, CO, NC], F32, tag="x")
                    eng = nc.sync if (j % 2 == 0) else nc.scalar
                    eng.dma_start(xa[:], xv[:, :, j * NC:(j + 1) * NC])
                    h = hp.tile([P, CO, NC], F32R, tag="h")
                    for co in range(CO):
                        nc.vector.tensor_scalar(
                            h[:, co], xa[:, co],
                            a_sb[:, co, b:b + 1], c_sb[:, co, b:b + 1],
                            ALU.mult, ALU.add)
                    v = vp.tile([P, CO, NC], F32R, tag="v")
                    for ot in range(CO):
                        ps_v = cps.tile([P, NC], F32, tag="c")
                        for co in range(CO):
                            nc.tensor.matmul(
                                ps_v[:], wv_sb[:, co, ot * P:(ot + 1) * P],
                                h[:, co], start=(co == 0), stop=(co == CO - 1))
                        nc.scalar.activation(v[:, ot], ps_v[:], ACT.Identity,
                                             bias=vb_sb[:, ot:ot + 1])
                    if debug and b == 0 and j == 0:
                        nc.gpsimd.dma_start(dbg["dbg_v"][:], v[:])
                    return xa, v

                def finish(b, j, xa, v):
                    av = avp.tile([P, CO, NC], F32R, tag="av")
                    for ot in range(CO):
                        ps_a = cps.tile([P, NC], F32, tag="c")
                        nc.tensor.matmul(ps_a[:], abd_r[:, b * 4 + ot],
                                         v[:, ot], start=True, stop=True)
                        if ot % 2 == 0:
                            nc.scalar.copy(av[:, ot], ps_a[:])
                        else:
                            nc.vector.tensor_copy(av[:, ot], ps_a[:])
                    if debug and b == 0 and j == 0:
                        nc.gpsimd.dma_start(dbg["dbg_av"][:], av[:])
                    yv = yout[b].rearrange("(co ci) n -> ci co n", ci=P)
                    y_sb = yp.tile([P, CO, NC], F32, tag="y")
                    for ot in range(CO):
                        ps_o = cps.tile([P, NC], F32, tag="c")
                        for co in range(CO):
                            nc.tensor.matmul(
                                ps_o[:], wo_sb[:, co, ot * P:(ot + 1) * P],
                                av[:, co], start=(co == 0), stop=False)
                        # out bias as a rank-1 accumulation, then +residual
                        nc.tensor.matmul(ps_o[:], obt4[:, ot], ones4[:],
                                         start=False, stop=True,
                                         skip_group_check=True)
                        nc.vector.tensor_tensor(
                            y_sb[:, ot], ps_o[:], xa[:, ot], ALU.add)
                    eng = nc.sync if (j % 2 == 0) else nc.scalar
                    eng.dma_start(yv[:, :, j * NC:(j + 1) * NC], y_sb[:])

                softmax_b(0)
                for b in range(B):
                    pend = []
                    for j in range(nchunks):
                        pend.append((j, *emit_v(b, j)))
                    tr_attn(b)
                    if b + 1 < B:
                        # prefetch next batch's softmax so its attn
                        # transposes don't stall the PE
                        softmax_b(b + 1)
                    for j, xa, v in pend:
                        finish(b, j, xa, v)
                if debug:
                    nc.sync.dma_start(dbg["dbg_attn"][:], attn_sb[:])

    return nc


_WAITSPLIT_COUNTER = [0]


def _split_waits(nc, limit: int = 1):
    """Walrus in this container rejects instructions with more than one sync
    wait; split extras onto injected NoOps on the same engine."""
    n_split = 0
    for fn in nc.m.functions:
        for bb in fn.blocks:
            insts = list(bb.instructions)
            out = []
            changed = False
            for inst in insts:
                si = inst.sync_info
                waits = list(si.on_wait) if si is not None and si.on_wait \
                    else []
                if len(waits) > limit:
                    keep = waits[-limit:]
                    extra = waits[:-limit]
                    for i in range(0, len(extra), limit):
                        chunk = extra[i:i + limit]
                        _WAITSPLIT_COUNTER[0] += 1
                        nop = mybir.InstNoOp(
                            name=f"waitsplit-{_WAITSPLIT_COUNTER[0]}",
                            ins=[], outs=[])
                        nop.engine = inst.engine
                        nop.sync_info = mybir.SyncInfo(
                            on_wait=chunk, on_update=[])
                        out.append(nop)
                    si.on_wait = keep
                    n_split += 1
                    changed = True
                out.append(inst)
            if changed:
                bb.instructions = out
    return n_split


_CACHE = {}


def _get_module(n_loc, split=True, debug=False):
    key = (n_loc, split, debug)
    if key not in _CACHE:
        nc = build_module(n_loc, debug=debug)
        if split:
            _split_waits(nc, limit=1)
        _CACHE[key] = nc
    return _CACHE[key]


def make_in_maps(inputs, n_loc=None):
    x = np.ascontiguousarray(np.asarray(inputs["x"], dtype=np.float32))
    qkv_w = np.asarray(inputs["qkv_w"], dtype=np.float32)
    qkv_b = np.asarray(inputs["qkv_b"], dtype=np.float32)
    out_w = np.asarray(inputs["out_w"], dtype=np.float32)
    out_b = np.asarray(inputs["out_b"], dtype=np.float32)
    gn_w = np.asarray(inputs["gn_weight"], dtype=np.float32)
    gn_b = np.asarray(inputs["gn_bias"], dtype=np.float32)

    n_tot = int(np.prod(x.shape[2:]))
    if n_loc is None:
        n_loc = n_tot // N_CORES
    xf = x.reshape(B, C, n_tot)

    wqk_t = np.ascontiguousarray(_round_tf32(qkv_w[0:2 * C].T))
    wv_t = np.ascontiguousarray(_round_tf32(qkv_w[2 * C:3 * C].T))
    wo_t = np.ascontiguousarray(_round_tf32(out_w.T))
    qkb = np.ascontiguousarray(qkv_b[0:2 * C].reshape(1, 2 * C))
    vb2 = np.ascontiguousarray(qkv_b[2 * C:3 * C].reshape(CO, P).T)
    ob2 = np.ascontiguousarray(out_b.reshape(CO, P).T)
    gnw2 = np.ascontiguousarray(gn_w.reshape(CO, P).T)
    gnb2 = np.ascontiguousarray(gn_b.reshape(CO, P).T)

    shared = dict(wqk_t=wqk_t, wv_t=wv_t, wo_t=wo_t, qkb=qkb, vb2=vb2,
                  ob2=ob2, gnw2=gnw2, gnb2=gnb2)
    in_maps = []
    for c in range(N_CORES):
        sl = np.ascontiguousarray(xf[:, :, c * n_loc:(c + 1) * n_loc])
        in_maps.append({"xin": sl, **shared})
    return in_maps


def run(inputs, n_loc=None, debug=False, **kw):
    x = np.asarray(inputs["x"])
    n_tot = int(np.prod(x.shape[2:]))
    if n_loc is None:
        n_loc = n_tot // N_CORES
    nc = _get_module(n_loc, debug=debug)
    in_maps = make_in_maps(inputs, n_loc)
    res = bass_utils.run_bass_kernel_spmd(
        nc, in_maps, core_ids=list(range(N_CORES)), **kw)
    y = np.concatenate([res.results[c]["yout"] for c in range(N_CORES)],
                       axis=2)
    return y, res


def kernel(**inputs) -> np.ndarray:
    x = np.asarray(inputs["x"])
    y, _ = run(inputs)
    return y.reshape(x.shape).astype(np.asarray(x).dtype)


# revision 107
# speedup vs baseline: 1.1531x; 1.0258x over previous
"""AttentionBlock3D on 8 Trainium2 NeuronCores — Gram-matrix restructure.

Math (see reference.py): GroupNorm(8 groups) -> qkv 1x1 conv -> channel
attention (contract over tokens N, softmax over last d=64) -> out proj ->
residual.

Sharding: N = T*H*W = 16384 tokens split 8 ways (2048/core); every core
holds all 4 batches of its token slice.  Cross-core state: one GroupNorm
stats AllReduce (16 KB, all batches merged) + one logits AllReduce per
batch (128 KB each).

Key identity: the channel-attention logits contract over tokens, so
  L = Q K^T = Aq Gx Ak^T + uq bk~^T + bq~ uk^T + N bq~ bk~^T
with Gx = sum_n x x^T (Gram of RAW x — independent of GroupNorm stats!),
Aq = Wq diag(a), uq = Aq Sx, bq~ = Wq c + bq (a, c = GN scale/shift).
Gx is computed on the PE from t~4us (PE-transposed x tiles), entirely
overlapping the stats pipeline + AllReduce; the remaining per-batch work
(Y = Gx Ak^T, L = Aq Y, rank-1 corrections) is tiny.  Per-core PE work
drops from ~300us (baseline QK-projection path) to ~240us and there is
no startup bubble.

Phases (per-core):
  A: x streamed once ([128 x 2048] tiles, 4 DMA queues); bn_stats (DVE)
     + Identity/Square accum (ACT) -> per-channel sum/sumsq; ONE stats
     AllReduce for all batches; PE meanwhile transposes x blocks and
     accumulates Gx(b) per batch in PSUM.
  B: per batch: aqkT = diag(a)*[Wq|Wk]^T (DVE row-scale), Y = Gx aqkT_k,
     L = aqkT_q^T Y (+ rank-1 bias/GN corrections in fp32), extract the 8
     diagonal 64x64 head blocks -> per-batch logits AllReduce.
  C: per batch: softmax + PE blockdiag-transpose, then per 512-token
     chunk: h = a*x+c (DVE), V proj, attn@v, out proj, +residual, store.

All big matmuls run in float32r (free dim 512 -> 1 cycle/row); the rank-1
correction matmuls and transposes are exact fp32.
"""

import numpy as np

import concourse.bass as bass
import concourse.mybir as mybir
import concourse.tile as tile
from concourse import bass_utils

F32 = mybir.dt.float32
F32R = mybir.dt.float32r
AX = mybir.AxisListType.X
ALU = mybir.AluOpType
ACT = mybir.ActivationFunctionType

N_CORES = 8
B, C, T, H, W = 4, 512, 16, 32, 32
N_TOT = T * H * W            # 16384
NH, D = 8, 64                # heads, head dim
G = 8                        # groupnorm groups
EPS = 1e-5
P = 128
CO = C // P                  # 4 channel chunks
NC = 512                     # phase-C token chunk size


def _round_tf32(a: np.ndarray) -> np.ndarray:
    """Round fp32 to fp32r (keep 10 explicit mantissa bits, RNE)."""
    u = a.astype(np.float32).view(np.uint32).astype(np.uint64)
    u = (u + 0x1000 + ((u >> 13) & 1)) & 0xFFFFE000
    return u.astype(np.uint32).view(np.float32)


def build_module(n_loc: int, debug: bool = False):
    NT = n_loc // P              # token blocks per batch (16)
    nchunks = n_loc // NC        # phase-C chunks per batch (4)
    ngr = n_loc // 512           # bn_stats groups per tile
    ntot = n_loc * N_CORES
    m_group = (C // G) * ntot    # elements per (b, group) stat
    scale = float(D) ** -0.5

    nc = bass.Bass("TRN2", target_bir_lowering=False, debug=False,
                   num_devices=N_CORES)

    xin = nc.dram_tensor("xin", [B, C, n_loc], F32, kind="ExternalInput").ap()
    wqk_t = nc.dram_tensor("wqk_t", [C, 2 * C], F32, kind="ExternalInput").ap()
    wv_t = nc.dram_tensor("wv_t", [C, C], F32, kind="ExternalInput").ap()
    wo_t = nc.dram_tensor("wo_t", [C, C], F32, kind="ExternalInput").ap()
    qkb = nc.dram_tensor("qkb", [1, 2 * C], F32, kind="ExternalInput").ap()
    vb2 = nc.dram_tensor("vb2", [P, CO], F32, kind="ExternalInput").ap()
    ob2 = nc.dram_tensor("ob2", [P, CO], F32, kind="ExternalInput").ap()
    gnw2 = nc.dram_tensor("gnw2", [P, CO], F32, kind="ExternalInput").ap()
    gnb2 = nc.dram_tensor("gnb2", [P, CO], F32, kind="ExternalInput").ap()
    yout = nc.dram_tensor("yout", [B, C, n_loc], F32, kind="ExternalOutput").ap()
    dbg = {}
    if debug:
        for nm, shp in [("dbg_stats", [P, 2, CO, B]), ("dbg_a", [P, CO, B]),
                        ("dbg_c", [P, CO, B]), ("dbg_gx", [P, CO, C]),
                        ("dbg_logits", [P, B, 4, D]),
                        ("dbg_attn", [P, B, 4, D]),
                        ("dbg_vec", [2, B, 2 * C]),
                        ("dbg_v", [P, CO, NC]), ("dbg_av", [P, CO, NC])]:
            dbg[nm] = nc.dram_tensor(nm, shp, F32, kind="ExternalOutput").ap()

    from concourse.masks import make_identity
    from bass_rust import add_dep_helper as _adh

    with tile.TileContext(nc) as tc:
        with (
            tc.tile_pool(name="persist", bufs=1) as pers,
            tc.tile_pool(name="wvpool", bufs=1) as wvp,
            tc.tile_pool(name="dram", bufs=1, space="DRAM") as dram,
        ):
            # ------------- persistent tiles (consts on gpsimd queue so
            # the HWDGE queues start streaming x at t=0) -------------
            qkb_sb = pers.tile([1, 2 * C], F32)
            vb_sb = pers.tile([P, CO], F32)
            ob_sb = pers.tile([P, CO], F32)
            gnw_sb = pers.tile([P, CO], F32)
            gnb_sb = pers.tile([P, CO], F32)

            ident = pers.tile([P, P], F32)
            make_identity(nc, ident[:])
            sel_sb = pers.tile([P, 2], F32)
            nc.vector.memset(sel_sb[:], 0.0)
            nc.vector.memset(sel_sb[0:64, 0:1], 1.0)
            nc.vector.memset(sel_sb[64:128, 1:2], 1.0)
            selt_sb = pers.tile([2, P], F32)
            with tc.tile_pool(name="selps", bufs=1, space="PSUM") as selps:
                sel_pt = selps.tile([2, P], F32)
                nc.tensor.transpose(sel_pt[:], sel_sb[:], ident[:])
                nc.vector.tensor_copy(selt_sb[:], sel_pt[:])

            a_sb = pers.tile([P, CO, B], F32)     # GN scale per (ci,co,b)
            c_sb = pers.tile([P, CO, B], F32)     # GN shift
            logits_sb = pers.tile([P, B, 4, D], F32)
            lg_full = pers.tile([P, B, 4, D], F32)

            stats = pers.tile([P, 2, CO, B], F32)   # local sum/sumsq
            stg = pers.tile([P, 2, CO, B], F32)     # global (post-AR)
            bst = pers.tile([P, B, CO, ngr, 6], F32)
            st_in = dram.tile([P, 2, CO, B], F32, name="st_in")
            st_gout = dram.tile([N_CORES, P, 2, CO, B], F32, name="st_gout")
            lg_in_l = [dram.tile([P, 2, 4, D], F32, name=f"lg_in{pp}")
                       for pp in range(B // 2)]
            lg_out_l = [dram.tile([P, 2, 4, D], F32, name=f"lg_out{pp}")
                        for pp in range(B // 2)]

            # rank-1 correction vectors (uqk/8, bqk, N*bqk/8) per batch, in
            # bf16 (the corrections are small relative to the logits, and
            # bf16 halves partition-0 SBUF pressure + runs 1 cycle/row)
            BF16 = mybir.dt.bfloat16
            cvec = pers.tile([1, 3, B, 2 * C], BF16)
            uqk8_l = [cvec[0:1, 0, bb] for bb in range(B)]
            bqk_l = [cvec[0:1, 1, bb] for bb in range(B)]
            nbq8_l = [cvec[0:1, 2, bb] for bb in range(B)]

            eps_t = pers.tile([2, 1], F32)
            nc.vector.memset(eps_t[:], EPS)

            # ================= phase A/B =================
            import contextlib
            ab_stack = contextlib.ExitStack()
            xp = ab_stack.enter_context(tc.tile_pool(name="xp", bufs=4))
            xs3p = ab_stack.enter_context(tc.tile_pool(name="xs3p", bufs=4))
            xtp = ab_stack.enter_context(tc.tile_pool(name="xtp", bufs=3))
            wqp = ab_stack.enter_context(tc.tile_pool(name="wqp", bufs=1))
            gxsb = ab_stack.enter_context(tc.tile_pool(name="gxsb", bufs=3))
            ysb = ab_stack.enter_context(tc.tile_pool(name="ysb", bufs=1))
            small = ab_stack.enter_context(tc.tile_pool(name="small", bufs=1))
            tps = ab_stack.enter_context(
                tc.tile_pool(name="tps", bufs=2, space="PSUM"))
            gxps = ab_stack.enter_context(
                tc.tile_pool(name="gxps", bufs=1, space="PSUM"))
            ylps = ab_stack.enter_context(
                tc.tile_pool(name="ylps", bufs=2, space="PSUM"))

            # x is loaded twice in phase A, as [P, 2, *] co-PAIR tiles (one
            # DMA per pair — per-DMA overhead is ~2.2us so bigger is better)
            # on the two HWDGE queues (SP: co 0/1, ACT: co 2/3); the gpsimd
            # queue stays free so the stats collective fires immediately:
            #  - transpose feed: [P, 2, 1024] halves (PE-paced)
            #  - stats feed: [P, 2, 2048], consumed right away (DVE
            #    bn_stats for co 0/1, ACT accum passes for co 2/3)
            qpair = [(nc.sync, 0), (nc.scalar, 2)]
            NJ = 2                      # transpose halves per (b, pair)
            gh = ngr // NJ              # bn_stats windows per half per co
            xa_t = {}

            def load_t(b, stats_here=True):
                # single x pass: [P, 2, 1024] co-pair halves feed BOTH the
                # PE transposes and (for b<3) the DVE bn_stats
                w = n_loc // NJ
                for jj in range(NJ):
                    for q, (eng, cb) in enumerate(qpair):
                        xa = xp.tile([P, 2, w], F32, tag=f"xt{q}",
                                     name=f"xt{b}_{q}_{jj}")
                        eng.dma_start(
                            xa[:],
                            xin[b, cb * P:(cb + 2) * P, jj * w:(jj + 1) * w]
                            .rearrange("(co ci) n -> ci co n", ci=P))
                        xa_t[(b, q, jj)] = xa
                        if stats_here:
                            for cc in range(2):
                                for g in range(gh):
                                    nc.vector.bn_stats(
                                        bst[:, b, cb + cc, jj * gh + g],
                                        xa[:, cc, g * 512:(g + 1) * 512])
                if stats_here:
                    stats_aggr(b)

            def stats_aggr(b):
                for co in range(CO):
                    mvt = small.tile([P, 2], F32, tag="mvt")
                    nc.vector.bn_aggr(mvt[:], bst[:, b, co])
                    nc.vector.tensor_scalar_mul(
                        stats[:, 0, co, b:b + 1], mvt[:, 0:1], float(n_loc))
                    nc.vector.tensor_tensor(
                        stats[:, 1, co, b:b + 1], mvt[:, 0:1],
                        mvt[:, 0:1], ALU.mult)
                    nc.vector.tensor_tensor(
                        stats[:, 1, co, b:b + 1],
                        stats[:, 1, co, b:b + 1], mvt[:, 1:2],
                        ALU.add)
                    nc.vector.tensor_scalar_mul(
                        stats[:, 1, co, b:b + 1],
                        stats[:, 1, co, b:b + 1], float(n_loc))

            # ---- PE: transpose x + accumulate Gram, software-pipelined
            # with lag so the ACT evacuation never stalls the PE ----
            TGX_LAG = 2
            gx_ps = {}
            gsb_l = {}

            def _emit_t(b, tb):
                tpb = NT // NJ          # tok-blocks per transpose half
                pt = tps.tile([P, C], F32, tag="t")
                for co in range(CO):
                    q, cc = divmod(co, 2)
                    nc.tensor.transpose(
                        pt[:, co * P:(co + 1) * P],
                        xa_t[(b, q, tb // tpb)][:, cc,
                                                (tb % tpb) * P:
                                                (tb % tpb + 1) * P],
                        ident[:])
                xt = xtp.tile([P, C], F32R, tag="xt")
                nc.scalar.copy(xt[:], pt[:])
                return xt

            def _emit_gx(b, tb, xt):
                if tb == 0:
                    gx_ps[b] = [gxps.tile([P, C], F32, tag=f"gx{co}",
                                          name=f"gx{b}_{co}")
                                for co in range(CO)]
                for co in range(CO):
                    nc.tensor.matmul(
                        gx_ps[b][co][:], xt[:, co * P:(co + 1) * P],
                        xt[:], start=(tb == 0), stop=(tb == NT - 1))
                if tb == NT - 1:
                    # evacuate Gram to SBUF (ACT — DVE runs bn_stats and
                    # must not serialize the Gram pipeline behind them)
                    gsb = gxsb.tile([P, CO, C], F32R, tag="gx",
                                    name=f"gxsb{b}")
                    for co in range(CO):
                        nc.scalar.copy(gsb[:, co], gx_ps[b][co][:])
                    if debug and b == 0:
                        nc.gpsimd.dma_start(dbg["dbg_gx"][:], gsb[:])
                    gsb_l[b] = gsb

            def t_gx_run(batches, steps=None):
                if steps is None:
                    steps = [(b, tb) for b in batches for tb in range(NT)]
                xts = {}
                for i, (b, tb) in enumerate(steps):
                    xts[i] = _emit_t(b, tb)
                    if i >= TGX_LAG:
                        bb, tt = steps[i - TGX_LAG]
                        _emit_gx(bb, tt, xts.pop(i - TGX_LAG))
                for i in range(len(steps) - TGX_LAG, len(steps)):
                    bb, tt = steps[i]
                    _emit_gx(bb, tt, xts.pop(i))

            # ---- stats AllGather (cheaper than AllReduce in the fabric:
            # no reduce pass) + local 8-way sum + all-batch GN prep ----
            def stats_ar():
                nc.gpsimd.dma_start(st_in[:], stats[:])
                nc.gpsimd.collective_compute(
                    "AllGather", ALU.bypass,
                    replica_groups=[list(range(N_CORES))],
                    ins=[st_in.opt()], outs=[st_gout.opt()],
                )
                stg8 = pers.tile([P, N_CORES, 2 * CO * B], F32)
                nc.gpsimd.dma_start(
                    stg8[:], st_gout.rearrange("g p a c b -> p g (a c b)"))
                nc.vector.reduce_sum(
                    stg[:].rearrange("p a c b -> p (a c b)"),
                    stg8[:].rearrange("p g a -> p a g"), AX)

            def prep_all(ps_pool):
                # group stats for all batches in one go: [2, CO, B]
                nf = 2 * CO * B
                pt1 = ps_pool.tile([P, C], F32, tag="yl", name="prep_ps")
                nc.tensor.matmul(
                    pt1[0:2, 0:nf], sel_sb[:],
                    stg[:].rearrange("p a b c -> p (a b c)"),
                    start=True, stop=True, skip_group_check=True)
                gst = small.tile([2, 2, CO, B], F32, tag="gst")
                nc.vector.tensor_copy(
                    gst[:].rearrange("p a b c -> p (a b c)"), pt1[0:2, 0:nf])
                mean_t = small.tile([2, CO, B], F32, tag="mean")
                nc.vector.tensor_scalar_mul(mean_t[:], gst[:, 0],
                                            1.0 / m_group)
                ex2_t = small.tile([2, CO, B], F32, tag="ex2")
                nc.vector.tensor_scalar_mul(ex2_t[:], gst[:, 1], 1.0 / m_group)
                var_t = small.tile([2, CO, B], F32, tag="var")
                nc.vector.tensor_tensor(var_t[:], mean_t[:], mean_t[:],
                                        ALU.mult)
                nc.vector.tensor_tensor(var_t[:], ex2_t[:], var_t[:],
                                        ALU.subtract)
                rstd_t = small.tile([2, CO, B], F32, tag="rstd")
                nc.scalar.activation(rstd_t[:], var_t[:], ACT.Sqrt,
                                     bias=eps_t[:])
                nc.vector.reciprocal(rstd_t[:], rstd_t[:])
                cg_t = small.tile([2, CO, B], F32, tag="cg")
                nc.vector.tensor_tensor(cg_t[:], mean_t[:], rstd_t[:],
                                        ALU.mult)
                nc.vector.tensor_scalar_mul(cg_t[:], cg_t[:], -1.0)
                rc2 = small.tile([2, 2, CO, B], F32, tag="rc2")
                nc.vector.tensor_copy(rc2[:, 0], rstd_t[:])
                nc.vector.tensor_copy(rc2[:, 1], cg_t[:])
                nc.tensor.matmul(
                    pt1[:, 512 - nf:512], selt_sb[:],
                    rc2[:].rearrange("p a b c -> p (a b c)"),
                    start=True, stop=True, skip_group_check=True)
                bc = small.tile([P, 2, CO, B], F32, tag="bc")
                nc.vector.tensor_copy(
                    bc[:].rearrange("p a b c -> p (a b c)"),
                    pt1[:, 512 - nf:512])
                # a = rstd*gnw, c = (-mean*rstd)*gnw + gnb, per batch
                for b in range(B):
                    nc.vector.tensor_tensor(a_sb[:, :, b], bc[:, 0, :, b],
                                            gnw_sb[:], ALU.mult)
                    nc.vector.tensor_tensor(c_sb[:, :, b], bc[:, 1, :, b],
                                            gnw_sb[:], ALU.mult)
                    nc.vector.tensor_tensor(c_sb[:, :, b], c_sb[:, :, b],
                                            gnb_sb[:], ALU.add)

            # ---- per-batch: correction vectors via PE ----
            def vec_prep(b, ps_pool):
                pv = small.tile([P, CO, 2], F32R, tag="pv")
                for co in range(CO):
                    nc.vector.tensor_tensor(pv[:, co, 0:1], a_sb[:, co, b:b+1],
                                            stg[:, 0, co, b:b + 1], ALU.mult)
                    nc.vector.tensor_copy(pv[:, co, 1:2], c_sb[:, co, b:b+1])
                # u and W@c rows accumulate in separate PSUM banks, both
                # at partition 0 (single-partition ops must sit at base 0)
                for half in range(2):
                    vpu = ps_pool.tile([P, C], F32, tag="yl",
                                       name=f"vecu{b}_{half}")
                    vpc = ps_pool.tile([P, C], F32, tag="yl",
                                       name=f"vecc{b}_{half}")
                    sl = slice(half * 512, half * 512 + 512)
                    for co in range(CO):
                        nc.tensor.matmul(
                            vpu[0:1, 0:C], pv[:, co, 0:1], wqk_sb[:, co, sl],
                            start=(co == 0), stop=(co == CO - 1),
                            skip_group_check=True)
                        nc.tensor.matmul(
                            vpc[0:1, 0:C], pv[:, co, 1:2], wqk_sb[:, co, sl],
                            start=(co == 0), stop=(co == CO - 1),
                            skip_group_check=True)
                    # u = A@Sx -> uqk8; bqk = W@c + qkv bias (from PSUM)
                    nc.vector.tensor_scalar_mul(uqk8_l[b][:, sl],
                                                vpu[0:1, :], 1.0 / N_CORES)
                    nc.vector.tensor_tensor(bqk_l[b][:, sl], vpc[0:1, :],
                                            qkb_sb[:, sl], ALU.add)
                # nbq8 = N*bqk/8
                nc.vector.tensor_scalar_mul(nbq8_l[b][:], bqk_l[b][:],
                                            float(ntot) / N_CORES)
                if debug:
                    nc.sync.dma_start(dbg["dbg_vec"][:, b], ub[:])

            # ---- per-batch: Y = (a.Gx) Wk^T, Y' = a.Y, L = Wq^T Y' ----
            # both diag(a) GN scales fold into the evacuations: an in-place
            # DVE row-scale of the Gram and an ACT scale-copy of Y
            def y_l(b, gsb):
                for co in range(CO):
                    nc.vector.tensor_scalar_mul(gsb[:, co], gsb[:, co],
                                                a_sb[:, co, b:b + 1])
                y_sb = ysb.tile([P, CO, C], F32R, tag="y", name=f"ysb{b}")
                for c1 in range(CO):
                    yp = ylps.tile([P, C], F32, tag="yl", name=f"y{b}_{c1}")
                    for c2 in range(CO):
                        nc.tensor.matmul(
                            yp[:], gsb[:, c2, c1 * P:(c1 + 1) * P],
                            wqk_sb[:, c2, C:2 * C],
                            start=(c2 == 0), stop=(c2 == CO - 1))
                    if c1 % 2 == 0:
                        nc.scalar.activation(y_sb[:, c1], yp[:],
                                             ACT.Identity,
                                             scale=a_sb[:, c1, b:b + 1])
                    else:
                        nc.vector.tensor_scalar_mul(y_sb[:, c1], yp[:],
                                                    a_sb[:, c1, b:b + 1])
                last_mm = None
                for dc in range(CO):
                    lp = ylps.tile([P, C], F32, tag="yl", name=f"l{b}_{dc}")
                    for c1 in range(CO):
                        nc.tensor.matmul(
                            lp[:], wqk_sb[:, c1, dc * P:(dc + 1) * P],
                            y_sb[:, c1], start=(c1 == 0), stop=False,
                            skip_group_check=True)
                    # rank-1 corrections on the two diagonal head blocks
                    for par in range(2):
                        hh = 2 * dc + par
                        rows = slice(par * 64, par * 64 + 64)
                        cols = slice(hh * 64, hh * 64 + 64)
                        tp = (0, 64) if par else None
                        ksl = slice(C + hh * 64, C + hh * 64 + 64)
                        qsl = slice(hh * 64, hh * 64 + 64)
                        nc.tensor.matmul(
                            lp[rows, cols], uqk8_l[b][:, qsl],
                            bqk_l[b][:, ksl], start=False, stop=False,
                            tile_position=tp, skip_group_check=True)
                        nc.tensor.matmul(
                            lp[rows, cols], bqk_l[b][:, qsl],
                            uqk8_l[b][:, ksl], start=False, stop=False,
                            tile_position=tp, skip_group_check=True)
                        last_mm = nc.tensor.matmul(
                            lp[rows, cols], nbq8_l[b][:, qsl],
                            bqk_l[b][:, ksl], start=False, stop=(par == 1),
                            tile_position=tp, skip_group_check=True)
                    # extract diagonal head blocks (alternate DVE/ACT to
                    # halve the serialized per-op SEQ latency chain)
                    for par in range(2):
                        hh = 2 * dc + par
                        rows = slice(par * 64, par * 64 + 64)
                        if par == 0:
                            nc.scalar.copy(
                                logits_sb[rows, b, dc, :],
                                lp[rows, hh * 64:hh * 64 + 64])
                        else:
                            nc.vector.tensor_copy(
                                logits_sb[rows, b, dc, :],
                                lp[rows, hh * 64:hh * 64 + 64])
                return last_mm

            def lg_ar_in(p):
                # paired logits AllReduce: batches 2p, 2p+1 in one 256KB op
                nc.gpsimd.dma_start(lg_in_l[p][:],
                                    logits_sb[:, 2 * p:2 * p + 2])
                nc.gpsimd.collective_compute(
                    "AllReduce", ALU.add,
                    replica_groups=[list(range(N_CORES))],
                    ins=[lg_in_l[p].opt()], outs=[lg_out_l[p].opt()],
                )

            def lg_ar_out(p):
                # emitted after BOTH collectives so the copy-back of pair 0
                # does not block pair 1's input DMA in the Pool FIFO
                nc.gpsimd.dma_start(lg_full[:, 2 * p:2 * p + 2],
                                    lg_out_l[p][:])

            # ============ emit phase A/B in PE-schedule order ============
            # stats feed: batches 0/1/2 read the transpose tiles (b2's
            # bn_stats emitted LAST — its tiles arrive at PE pace); batch
            # 3 loads on the (otherwise idle) gpsimd queue early
            load_t(0, stats_here=True)
            load_t(1, stats_here=True)
            load_t(2, stats_here=False)
            load_t(3, stats_here=False)
            wh = n_loc // 2
            for jj in range(2):
                for q in range(2):
                    cb = 2 * q
                    xs = xs3p.tile([P, 2, wh], F32, tag="xs3")
                    nc.gpsimd.dma_start(
                        xs[:], xin[3, cb * P:(cb + 2) * P,
                                   jj * wh:(jj + 1) * wh]
                        .rearrange("(co ci) n -> ci co n", ci=P))
                    for cc in range(2):
                        for g in range(gh):
                            nc.vector.bn_stats(
                                bst[:, 3, cb + cc, jj * gh + g],
                                xs[:, cc, g * 512:(g + 1) * 512])
            stats_aggr(3)
            for jj in range(NJ):
                for q, (eng, cb) in enumerate(qpair):
                    for cc in range(2):
                        for g in range(gh):
                            nc.vector.bn_stats(
                                bst[:, 2, cb + cc, jj * gh + g],
                                xa_t[(2, q, jj)][:, cc,
                                                 g * 512:(g + 1) * 512])
            stats_aggr(2)
            nc.gpsimd.dma_start(qkb_sb[:], qkb[:])
            nc.gpsimd.dma_start(vb_sb[:], vb2[:])
            nc.gpsimd.dma_start(ob_sb[:], ob2[:])
            nc.gpsimd.dma_start(gnw_sb[:], gnw2[:])
            nc.gpsimd.dma_start(gnb_sb[:], gnb2[:])
            # weights BEFORE st_in in the Pool FIFO: st_in waits for stats
            # anyway, and anything after it would delay the logits ARs
            wqk_sb = wqp.tile([P, CO, 2 * C], F32R)
            nc.gpsimd.dma_start(
                wqk_sb[:], wqk_t.rearrange("(co ci) o -> ci co o", ci=P))
            wv_sb = wvp.tile([P, CO, C], F32R)
            wo_sb = wvp.tile([P, CO, C], F32R)
            nc.gpsimd.dma_start(
                wv_sb[:], wv_t.rearrange("(co ci) o -> ci co o", ci=P))
            nc.gpsimd.dma_start(
                wo_sb[:], wo_t.rearrange("(co ci) o -> ci co o", ci=P))
            stats_ar()

            t_gx_run([0, 1, 2])
            # first quarter of batch 3's Gram fills the PE while the stats
            # collective completes; the rest resumes after YL0/YL1
            t_gx_run(None, steps=[(3, tb) for tb in range(NT // 4)])
            prep_all(ylps)
            vec_prep(0, ylps)
            vec_prep(1, ylps)
            y_l(0, gsb_l.pop(0))
            y_l(1, gsb_l.pop(1))
            lg_ar_in(0)
            vec_prep(2, ylps)
            vec_prep(3, ylps)
            t_gx_run(None, steps=[(3, tb) for tb in range(NT // 4, NT)])
            y_l(2, gsb_l.pop(2))
            y_l(3, gsb_l.pop(3))
            lg_ar_in(1)   # pair-1 input DMA precedes pair-0's copy-back in
            lg_ar_out(0)  # the Pool FIFO so AR23 starts the moment AR01 ends
            lg_ar_out(1)
            _ = gh  # silence lint; gh used by load_t

            if debug:
                nc.sync.dma_start(dbg["dbg_a"][:], a_sb[:])
                nc.sync.dma_start(dbg["dbg_c"][:], c_sb[:])
                nc.sync.dma_start(dbg["dbg_stats"][:], stg[:])
                nc.sync.dma_start(dbg["dbg_logits"][:], lg_full[:])
            ab_stack.close()

            # ================= phase C =================
            with (
                tc.tile_pool(name="cpers", bufs=1) as cpers,
                tc.tile_pool(name="xc", bufs=5) as xc,
                tc.tile_pool(name="hp", bufs=2) as hp,
                tc.tile_pool(name="vp", bufs=4) as vp,
                tc.tile_pool(name="avp", bufs=2) as avp,
                tc.tile_pool(name="yp", bufs=2) as yp,
                tc.tile_pool(name="smp", bufs=4) as smp,
                tc.tile_pool(name="cps", bufs=6, space="PSUM") as cps,
                tc.tile_pool(name="trp", bufs=2, space="PSUM") as trp,
            ):
                attn_sb = cpers.tile([P, B, 4, D], F32)
                abd_f = cpers.tile([P, B * 4, P], F32)   # block-diag attn
                abd_r = cpers.tile([P, B * 4, P], F32R)  # transposed, f32r
                nc.vector.memset(abd_f[:], 0.0)
                # transposed out-bias rows + ones row: the out bias is
                # added by a rank-1 PE matmul folded into the out-proj
                obt4 = cpers.tile([1, CO, P], mybir.dt.bfloat16)
                ones4 = cpers.tile([1, NC], mybir.dt.bfloat16)
                nc.vector.memset(ones4[:], 1.0)
                for ot in range(CO):
                    pt = trp.tile([P, P], F32, tag="pt", name=f"ob{ot}")
                    nc.tensor.transpose(pt[0:1, :], ob_sb[:, ot:ot + 1],
                                        ident[:])
                    nc.scalar.copy(obt4[:, ot], pt[0:1, :])

                def softmax_b(b):
                    for hp_i in range(4):
                        blk = lg_full[:, b, hp_i]
                        mx = smp.tile([P, 1], F32, tag="mx")
                        nc.vector.reduce_max(mx[:], blk, AX)
                        nbias = smp.tile([P, 1], F32, tag="nb")
                        nc.vector.tensor_scalar_mul(nbias[:], mx[:], -scale)
                        ex = attn_sb[:, b, hp_i]
                        nc.scalar.activation(ex, blk, ACT.Exp, bias=nbias[:],
                                             scale=scale)
                        sm = smp.tile([P, 1], F32, tag="sm")
                        nc.vector.reduce_sum(sm[:], ex, AX)
                        nc.vector.reciprocal(sm[:], sm[:])
                        nc.vector.tensor_scalar_mul(ex, ex, sm[:])
                        idx = b * 4 + hp_i
                        nc.vector.tensor_copy(abd_f[0:64, idx, 0:64],
                                              attn_sb[0:64, b, hp_i])
                        nc.vector.tensor_copy(abd_f[64:128, idx, 64:128],
                                              attn_sb[64:128, b, hp_i])

                def tr_attn(b):
                    for hp_i in range(4):
                        idx = b * 4 + hp_i
                        pt = trp.tile([P, P], F32, tag="pt")
                        nc.tensor.transpose(pt[:], abd_f[:, idx, :], ident[:])
                        nc.scalar.copy(abd_r[:, idx], pt[:])

                def emit_v(b, j):
                    xv = xin[b].rearrange("(co ci) n -> ci co n", ci=P)
                    xa = xc.tile([P, CO, NC], F32, tag="x")
                    eng = nc.sync if (j % 2 == 0) else nc.scalar
                    eng.dma_start(xa[:], xv[:, :, j * NC:(j + 1) * NC])
                    h = hp.tile([P, CO, NC], F32R, tag="h")
                    for co in range(CO):
                        nc.vector.tensor_scalar(
                            h[:, co], xa[:, co],
                            a_sb[:, co, b:b + 1], c_sb[:, co, b:b + 1],
                            ALU.mult, ALU.add)
                    v = vp.tile([P, CO, NC], F32R, tag="v")
                    for ot in range(CO):
                        ps_v = cps.tile([P, NC], F32, tag="c")
                        for co in range(CO):
                            nc.tensor.matmul(
                                ps_v[:], wv_sb[:, co, ot * P:(ot + 1) * P],
                                h[:, co], start=(co == 0), stop=(co == CO - 1))
                        if ot % 2 == 0:
                            nc.scalar.activation(v[:, ot], ps_v[:],
                                                 ACT.Identity,
                                                 bias=vb_sb[:, ot:ot + 1])
                        else:
                            nc.vector.tensor_scalar_add(
                                v[:, ot], ps_v[:], vb_sb[:, ot:ot + 1])
                    if debug and b == 0 and j == 0:
                        nc.gpsimd.dma_start(dbg["dbg_v"][:], v[:])
                    return xa, v

                def finish(b, j, xa, v):
                    av = avp.tile([P, CO, NC], F32R, tag="av")
                    for ot in range(CO):
                        ps_a = cps.tile([P, NC], F32, tag="c")
                        nc.tensor.matmul(ps_a[:], abd_r[:, b * 4 + ot],
                                         v[:, ot], start=True, stop=True)
                        if ot % 2 == 0:
                            nc.scalar.copy(av[:, ot], ps_a[:])
                        else:
                            nc.vector.tensor_copy(av[:, ot], ps_a[:])
                    if debug and b == 0 and j == 0:
                        nc.gpsimd.dma_start(dbg["dbg_av"][:], av[:])
                    yv = yout[b].rearrange("(co ci) n -> ci co n", ci=P)
                    y_sb = yp.tile([P, CO, NC], F32, tag="y")
                    for ot in range(CO):
                        ps_o = cps.tile([P, NC], F32, tag="c")
                        for co in range(CO):
                            nc.tensor.matmul(
                                ps_o[:], wo_sb[:, co, ot * P:(ot + 1) * P],
                                av[:, co], start=(co == 0), stop=False)
                        # out bias as a rank-1 accumulation, then +residual
                        nc.tensor.matmul(ps_o[:], obt4[:, ot], ones4[:],
                                         start=False, stop=True,
                                         skip_group_check=True)
                        nc.vector.tensor_tensor(
                            y_sb[:, ot], ps_o[:], xa[:, ot], ALU.add)
                    eng = nc.sync if (j % 2 == 0) else nc.scalar
                    eng.dma_start(yv[:, :, j * NC:(j + 1) * NC], y_sb[:])

                softmax_b(0)
                for b in range(B):
                    pend = []
                    for j in range(nchunks):
                        pend.append((j, *emit_v(b, j)))
                    tr_attn(b)
                    if b + 1 < B:
                        # prefetch next batch's softmax so its attn
                        # transposes don't stall the PE
                        softmax_b(b + 1)
                    for j, xa, v in pend:
                        finish(b, j, xa, v)
                if debug:
                    nc.sync.dma_start(dbg["dbg_attn"][:], attn_sb[:])

    return nc


_WAITSPLIT_COUNTER = [0]


def _split_waits(nc, limit: int = 1):
    """Walrus in this container rejects instructions with more than one sync
    wait; split extras onto injected NoOps on the same engine."""
    n_split = 0
    for fn in nc.m.functions:
        for bb in fn.blocks:
            insts = list(bb.instructions)
            out = []
            changed = False
            for inst in insts:
                si = inst.sync_info
                waits = list(si.on_wait) if si is not None and si.on_wait \
                    else []
                if len(waits) > limit:
                    keep = waits[-limit:]
                    extra = waits[:-limit]
                    for i in range(0, len(extra), limit):
                        chunk = extra[i:i + limit]
                        _WAITSPLIT_COUNTER[0] += 1
                        nop = mybir.InstNoOp(
                            name=f"waitsplit-{_WAITSPLIT_COUNTER[0]}",
                            ins=[], outs=[])
                        nop.engine = inst.engine
                        nop.sync_info = mybir.SyncInfo(
                            on_wait=chunk, on_update=[])
                        out.append(nop)
                    si.on_wait = keep
                    n_split += 1
                    changed = True
                out.append(inst)
            if changed:
                bb.instructions = out
    return n_split


_CACHE = {}


def _get_module(n_loc, split=True, debug=False):
    key = (n_loc, split, debug)
    if key not in _CACHE:
        nc = build_module(n_loc, debug=debug)
        if split:
            _split_waits(nc, limit=1)
        _CACHE[key] = nc
    return _CACHE[key]


def make_in_maps(inputs, n_loc=None):
    x = np.ascontiguousarray(np.asarray(inputs["x"], dtype=np.float32))
    qkv_w = np.asarray(inputs["qkv_w"], dtype=np.float32)
    qkv_b = np.asarray(inputs["qkv_b"], dtype=np.float32)
    out_w = np.asarray(inputs["out_w"], dtype=np.float32)
    out_b = np.asarray(inputs["out_b"], dtype=np.float32)
    gn_w = np.asarray(inputs["gn_weight"], dtype=np.float32)
    gn_b = np.asarray(inputs["gn_bias"], dtype=np.float32)

    n_tot = int(np.prod(x.shape[2:]))
    if n_loc is None:
        n_loc = n_tot // N_CORES
    xf = x.reshape(B, C, n_tot)

    wqk_t = np.ascontiguousarray(_round_tf32(qkv_w[0:2 * C].T))
    wv_t = np.ascontiguousarray(_round_tf32(qkv_w[2 * C:3 * C].T))
    wo_t = np.ascontiguousarray(_round_tf32(out_w.T))
    qkb = np.ascontiguousarray(qkv_b[0:2 * C].reshape(1, 2 * C))
    vb2 = np.ascontiguousarray(qkv_b[2 * C:3 * C].reshape(CO, P).T)
    ob2 = np.ascontiguousarray(out_b.reshape(CO, P).T)
    gnw2 = np.ascontiguousarray(gn_w.reshape(CO, P).T)
    gnb2 = np.ascontiguousarray(gn_b.reshape(CO, P).T)

    shared = dict(wqk_t=wqk_t, wv_t=wv_t, wo_t=wo_t, qkb=qkb, vb2=vb2,
                  ob2=ob2, gnw2=gnw2, gnb2=gnb2)
    in_maps = []
    for c in range(N_CORES):
        sl = np.ascontiguousarray(xf[:, :, c * n_loc:(c + 1) * n_loc])
        in_maps.append({"xin": sl, **shared})
    return in_maps


def run(inputs, n_loc=None, debug=False, **kw):
    x = np.asarray(inputs["x"])
    n_tot = int(np.prod(x.shape[2:]))
    if n_loc is None:
        n_loc = n_tot // N_CORES
    nc = _get_module(n_loc, debug=debug)
    in_maps = make_in_maps(inputs, n_loc)
    res = bass_utils.run_bass_kernel_spmd(
        nc, in_maps, core_ids=list(range(N_CORES)), **kw)
    y = np.concatenate([res.results[c]["yout"] for c in range(N_CORES)],
                       axis=2)
    return y, res


def kernel(**inputs) -> np.ndarray:
    x = np.asarray(inputs["x"])
    y, _ = run(inputs)
    return y.reshape(x.shape).astype(np.asarray(x).dtype)


# revision 110
# speedup vs baseline: 1.1662x; 1.0113x over previous
"""AttentionBlock3D on 8 Trainium2 NeuronCores — Gram-matrix restructure.

Math (see reference.py): GroupNorm(8 groups) -> qkv 1x1 conv -> channel
attention (contract over tokens N, softmax over last d=64) -> out proj ->
residual.

Sharding: N = T*H*W = 16384 tokens split 8 ways (2048/core); every core
holds all 4 batches of its token slice.  Cross-core state: one GroupNorm
stats AllReduce (16 KB, all batches merged) + one logits AllReduce per
batch (128 KB each).

Key identity: the channel-attention logits contract over tokens, so
  L = Q K^T = Aq Gx Ak^T + uq bk~^T + bq~ uk^T + N bq~ bk~^T
with Gx = sum_n x x^T (Gram of RAW x — independent of GroupNorm stats!),
Aq = Wq diag(a), uq = Aq Sx, bq~ = Wq c + bq (a, c = GN scale/shift).
Gx is computed on the PE from t~4us (PE-transposed x tiles), entirely
overlapping the stats pipeline + AllReduce; the remaining per-batch work
(Y = Gx Ak^T, L = Aq Y, rank-1 corrections) is tiny.  Per-core PE work
drops from ~300us (baseline QK-projection path) to ~240us and there is
no startup bubble.

Phases (per-core):
  A: x streamed once ([128 x 2048] tiles, 4 DMA queues); bn_stats (DVE)
     + Identity/Square accum (ACT) -> per-channel sum/sumsq; ONE stats
     AllReduce for all batches; PE meanwhile transposes x blocks and
     accumulates Gx(b) per batch in PSUM.
  B: per batch: aqkT = diag(a)*[Wq|Wk]^T (DVE row-scale), Y = Gx aqkT_k,
     L = aqkT_q^T Y (+ rank-1 bias/GN corrections in fp32), extract the 8
     diagonal 64x64 head blocks -> per-batch logits AllReduce.
  C: per batch: softmax + PE blockdiag-transpose, then per 512-token
     chunk: h = a*x+c (DVE), V proj, attn@v, out proj, +residual, store.

All big matmuls run in float32r (free dim 512 -> 1 cycle/row); the rank-1
correction matmuls and transposes are exact fp32.
"""

import numpy as np

import concourse.bass as bass
import concourse.mybir as mybir
import concourse.tile as tile
from concourse import bass_utils

F32 = mybir.dt.float32
F32R = mybir.dt.float32r
AX = mybir.AxisListType.X
ALU = mybir.AluOpType
ACT = mybir.ActivationFunctionType

N_CORES = 8
B, C, T, H, W = 4, 512, 16, 32, 32
N_TOT = T * H * W            # 16384
NH, D = 8, 64                # heads, head dim
G = 8                        # groupnorm groups
EPS = 1e-5
P = 128
CO = C // P                  # 4 channel chunks
NC = 512                     # phase-C token chunk size


def _round_tf32(a: np.ndarray) -> np.ndarray:
    """Round fp32 to fp32r (keep 10 explicit mantissa bits, RNE)."""
    u = a.astype(np.float32).view(np.uint32).astype(np.uint64)
    u = (u + 0x1000 + ((u >> 13) & 1)) & 0xFFFFE000
    return u.astype(np.uint32).view(np.float32)


def build_module(n_loc: int, debug: bool = False):
    NT = n_loc // P              # token blocks per batch (16)
    nchunks = n_loc // NC        # phase-C chunks per batch (4)
    ngr = n_loc // 512           # bn_stats groups per tile
    ntot = n_loc * N_CORES
    m_group = (C // G) * ntot    # elements per (b, group) stat
    scale = float(D) ** -0.5

    nc = bass.Bass("TRN2", target_bir_lowering=False, debug=False,
                   num_devices=N_CORES)

    xin = nc.dram_tensor("xin", [B, C, n_loc], F32, kind="ExternalInput").ap()
    wqk_t = nc.dram_tensor("wqk_t", [C, 2 * C], F32, kind="ExternalInput").ap()
    wv_t = nc.dram_tensor("wv_t", [C, C], F32, kind="ExternalInput").ap()
    wo_t = nc.dram_tensor("wo_t", [C, C], F32, kind="ExternalInput").ap()
    qkb = nc.dram_tensor("qkb", [1, 2 * C], F32, kind="ExternalInput").ap()
    vb2 = nc.dram_tensor("vb2", [P, CO], F32, kind="ExternalInput").ap()
    ob2 = nc.dram_tensor("ob2", [P, CO], F32, kind="ExternalInput").ap()
    gnw2 = nc.dram_tensor("gnw2", [P, CO], F32, kind="ExternalInput").ap()
    gnb2 = nc.dram_tensor("gnb2", [P, CO], F32, kind="ExternalInput").ap()
    yout = nc.dram_tensor("yout", [B, C, n_loc], F32, kind="ExternalOutput").ap()
    dbg = {}
    if debug:
        for nm, shp in [("dbg_stats", [P, 2, CO, B]), ("dbg_a", [P, CO, B]),
                        ("dbg_c", [P, CO, B]), ("dbg_gx", [P, CO, C]),
                        ("dbg_logits", [P, B, 4, D]),
                        ("dbg_attn", [P, B, 4, D]),
                        ("dbg_vec", [2, B, 2 * C]),
                        ("dbg_v", [P, CO, NC]), ("dbg_av", [P, CO, NC])]:
            dbg[nm] = nc.dram_tensor(nm, shp, F32, kind="ExternalOutput").ap()

    from concourse.masks import make_identity
    from bass_rust import add_dep_helper as _adh

    with tile.TileContext(nc) as tc:
        with (
            tc.tile_pool(name="persist", bufs=1) as pers,
            tc.tile_pool(name="wvpool", bufs=1) as wvp,
            tc.tile_pool(name="dram", bufs=1, space="DRAM") as dram,
        ):
            # ------------- persistent tiles (consts on gpsimd queue so
            # the HWDGE queues start streaming x at t=0) -------------
            qkb_sb = pers.tile([1, 2 * C], F32)
            vb_sb = pers.tile([P, CO], F32)
            ob_sb = pers.tile([P, CO], F32)
            gnw_sb = pers.tile([P, CO], F32)
            gnb_sb = pers.tile([P, CO], F32)

            ident = pers.tile([P, P], F32)
            make_identity(nc, ident[:])
            sel_sb = pers.tile([P, 2], F32)
            nc.vector.memset(sel_sb[:], 0.0)
            nc.vector.memset(sel_sb[0:64, 0:1], 1.0)
            nc.vector.memset(sel_sb[64:128, 1:2], 1.0)
            selt_sb = pers.tile([2, P], F32)
            with tc.tile_pool(name="selps", bufs=1, space="PSUM") as selps:
                sel_pt = selps.tile([2, P], F32)
                nc.tensor.transpose(sel_pt[:], sel_sb[:], ident[:])
                nc.vector.tensor_copy(selt_sb[:], sel_pt[:])

            a_sb = pers.tile([P, CO, B], F32)     # GN scale per (ci,co,b)
            c_sb = pers.tile([P, CO, B], F32)     # GN shift
            logits_sb = pers.tile([P, B, 4, D], F32)
            lg_full = pers.tile([P, B, 4, D], F32)

            stats = pers.tile([P, 2, CO, B], F32)   # local sum/sumsq
            stg = pers.tile([P, 2, CO, B], F32)     # global (post-AR)
            bst = pers.tile([P, B, CO, ngr, 6], F32)
            st_in = dram.tile([P, 2, CO, B], F32, name="st_in")
            st_gout = dram.tile([N_CORES, P, 2, CO, B], F32, name="st_gout")
            lg_in_l = [dram.tile([P, 2, 4, D], F32, name=f"lg_in{pp}")
                       for pp in range(B // 2)]
            lg_out_l = [dram.tile([P, 2, 4, D], F32, name=f"lg_out{pp}")
                        for pp in range(B // 2)]

            # rank-1 correction vectors (uqk/8, bqk, N*bqk/8) per batch, in
            # bf16 (the corrections are small relative to the logits, and
            # bf16 halves partition-0 SBUF pressure + runs 1 cycle/row)
            BF16 = mybir.dt.bfloat16
            cvec = pers.tile([1, 3, B, 2 * C], BF16)
            uqk8_l = [cvec[0:1, 0, bb] for bb in range(B)]
            bqk_l = [cvec[0:1, 1, bb] for bb in range(B)]
            nbq8_l = [cvec[0:1, 2, bb] for bb in range(B)]

            eps_t = pers.tile([2, 1], F32)
            nc.vector.memset(eps_t[:], EPS)

            # ================= phase A/B =================
            import contextlib
            ab_stack = contextlib.ExitStack()
            xp = ab_stack.enter_context(tc.tile_pool(name="xp", bufs=6))
            xtp = ab_stack.enter_context(tc.tile_pool(name="xtp", bufs=3))
            wqp = ab_stack.enter_context(tc.tile_pool(name="wqp", bufs=1))
            gxsb = ab_stack.enter_context(tc.tile_pool(name="gxsb", bufs=3))
            ysb = ab_stack.enter_context(tc.tile_pool(name="ysb", bufs=1))
            small = ab_stack.enter_context(tc.tile_pool(name="small", bufs=1))
            tps = ab_stack.enter_context(
                tc.tile_pool(name="tps", bufs=2, space="PSUM"))
            gxps = ab_stack.enter_context(
                tc.tile_pool(name="gxps", bufs=1, space="PSUM"))
            ylps = ab_stack.enter_context(
                tc.tile_pool(name="ylps", bufs=2, space="PSUM"))

            # x is loaded twice in phase A, as [P, 2, *] co-PAIR tiles (one
            # DMA per pair — per-DMA overhead is ~2.2us so bigger is better)
            # on the two HWDGE queues (SP: co 0/1, ACT: co 2/3); the gpsimd
            # queue stays free so the stats collective fires immediately:
            #  - transpose feed: [P, 2, 1024] halves (PE-paced)
            #  - stats feed: [P, 2, 2048], consumed right away (DVE
            #    bn_stats for co 0/1, ACT accum passes for co 2/3)
            qpair = [(nc.sync, 0), (nc.scalar, 2)]
            NJ = 2                      # transpose halves per (b, pair)
            gh = ngr // NJ              # bn_stats windows per half per co
            xa_t = {}

            def load_t(b, stats_here=True):
                # single x pass: [P, 2, 1024] co-pair halves feed BOTH the
                # PE transposes and (for b<3) the DVE bn_stats
                w = n_loc // NJ
                for jj in range(NJ):
                    for q, (eng, cb) in enumerate(qpair):
                        xa = xp.tile([P, 2, w], F32, tag=f"xt{q}",
                                     name=f"xt{b}_{q}_{jj}")
                        eng.dma_start(
                            xa[:],
                            xin[b, cb * P:(cb + 2) * P, jj * w:(jj + 1) * w]
                            .rearrange("(co ci) n -> ci co n", ci=P))
                        xa_t[(b, q, jj)] = xa
                        if stats_here:
                            for cc in range(2):
                                for g in range(gh):
                                    nc.vector.bn_stats(
                                        bst[:, b, cb + cc, jj * gh + g],
                                        xa[:, cc, g * 512:(g + 1) * 512])
                if stats_here:
                    stats_aggr(b)

            def stats_aggr(b):
                for co in range(CO):
                    mvt = small.tile([P, 2], F32, tag="mvt")
                    nc.vector.bn_aggr(mvt[:], bst[:, b, co])
                    nc.vector.tensor_scalar_mul(
                        stats[:, 0, co, b:b + 1], mvt[:, 0:1], float(n_loc))
                    nc.vector.tensor_tensor(
                        stats[:, 1, co, b:b + 1], mvt[:, 0:1],
                        mvt[:, 0:1], ALU.mult)
                    nc.vector.tensor_tensor(
                        stats[:, 1, co, b:b + 1],
                        stats[:, 1, co, b:b + 1], mvt[:, 1:2],
                        ALU.add)
                    nc.vector.tensor_scalar_mul(
                        stats[:, 1, co, b:b + 1],
                        stats[:, 1, co, b:b + 1], float(n_loc))

            # ---- PE: transpose x + accumulate Gram, software-pipelined
            # with lag so the ACT evacuation never stalls the PE ----
            TGX_LAG = 2
            gx_ps = {}
            gsb_l = {}

            def _emit_t(b, tb):
                tpb = NT // NJ          # tok-blocks per transpose half
                pt = tps.tile([P, C], F32, tag="t")
                for co in range(CO):
                    q, cc = divmod(co, 2)
                    nc.tensor.transpose(
                        pt[:, co * P:(co + 1) * P],
                        xa_t[(b, q, tb // tpb)][:, cc,
                                                (tb % tpb) * P:
                                                (tb % tpb + 1) * P],
                        ident[:])
                xt = xtp.tile([P, C], F32R, tag="xt")
                nc.scalar.copy(xt[:], pt[:])
                return xt

            def _emit_gx(b, tb, xt):
                if tb == 0:
                    gx_ps[b] = [gxps.tile([P, C], F32, tag=f"gx{co}",
                                          name=f"gx{b}_{co}")
                                for co in range(CO)]
                for co in range(CO):
                    nc.tensor.matmul(
                        gx_ps[b][co][:], xt[:, co * P:(co + 1) * P],
                        xt[:], start=(tb == 0), stop=(tb == NT - 1))
                if tb == NT - 1:
                    # evacuate Gram to SBUF (ACT — DVE runs bn_stats and
                    # must not serialize the Gram pipeline behind them)
                    gsb = gxsb.tile([P, CO, C], F32R, tag="gx",
                                    name=f"gxsb{b}")
                    for co in range(CO):
                        nc.scalar.copy(gsb[:, co], gx_ps[b][co][:])
                    if debug and b == 0:
                        nc.gpsimd.dma_start(dbg["dbg_gx"][:], gsb[:])
                    gsb_l[b] = gsb

            def t_gx_run(batches, steps=None):
                if steps is None:
                    steps = [(b, tb) for b in batches for tb in range(NT)]
                xts = {}
                for i, (b, tb) in enumerate(steps):
                    xts[i] = _emit_t(b, tb)
                    if i >= TGX_LAG:
                        bb, tt = steps[i - TGX_LAG]
                        _emit_gx(bb, tt, xts.pop(i - TGX_LAG))
                for i in range(len(steps) - TGX_LAG, len(steps)):
                    bb, tt = steps[i]
                    _emit_gx(bb, tt, xts.pop(i))

            # ---- stats AllGather (cheaper than AllReduce in the fabric:
            # no reduce pass) + local 8-way sum + all-batch GN prep ----
            def stats_ar():
                nc.gpsimd.dma_start(st_in[:], stats[:])
                nc.gpsimd.collective_compute(
                    "AllGather", ALU.bypass,
                    replica_groups=[list(range(N_CORES))],
                    ins=[st_in.opt()], outs=[st_gout.opt()],
                )
                stg8 = pers.tile([P, N_CORES, 2 * CO * B], F32)
                nc.gpsimd.dma_start(
                    stg8[:], st_gout.rearrange("g p a c b -> p g (a c b)"))
                nc.vector.reduce_sum(
                    stg[:].rearrange("p a c b -> p (a c b)"),
                    stg8[:].rearrange("p g a -> p a g"), AX)

            def prep_all(ps_pool):
                # group stats for all batches in one go: [2, CO, B]
                nf = 2 * CO * B
                pt1 = ps_pool.tile([P, C], F32, tag="yl", name="prep_ps")
                nc.tensor.matmul(
                    pt1[0:2, 0:nf], sel_sb[:],
                    stg[:].rearrange("p a b c -> p (a b c)"),
                    start=True, stop=True, skip_group_check=True)
                gst = small.tile([2, 2, CO, B], F32, tag="gst")
                nc.vector.tensor_copy(
                    gst[:].rearrange("p a b c -> p (a b c)"), pt1[0:2, 0:nf])
                mean_t = small.tile([2, CO, B], F32, tag="mean")
                nc.vector.tensor_scalar_mul(mean_t[:], gst[:, 0],
                                            1.0 / m_group)
                ex2_t = small.tile([2, CO, B], F32, tag="ex2")
                nc.vector.tensor_scalar_mul(ex2_t[:], gst[:, 1], 1.0 / m_group)
                var_t = small.tile([2, CO, B], F32, tag="var")
                nc.vector.tensor_tensor(var_t[:], mean_t[:], mean_t[:],
                                        ALU.mult)
                nc.vector.tensor_tensor(var_t[:], ex2_t[:], var_t[:],
                                        ALU.subtract)
                rstd_t = small.tile([2, CO, B], F32, tag="rstd")
                nc.scalar.activation(rstd_t[:], var_t[:], ACT.Sqrt,
                                     bias=eps_t[:])
                nc.vector.reciprocal(rstd_t[:], rstd_t[:])
                cg_t = small.tile([2, CO, B], F32, tag="cg")
                nc.vector.tensor_tensor(cg_t[:], mean_t[:], rstd_t[:],
                                        ALU.mult)
                nc.vector.tensor_scalar_mul(cg_t[:], cg_t[:], -1.0)
                rc2 = small.tile([2, 2, CO, B], F32, tag="rc2")
                nc.vector.tensor_copy(rc2[:, 0], rstd_t[:])
                nc.vector.tensor_copy(rc2[:, 1], cg_t[:])
                nc.tensor.matmul(
                    pt1[:, 512 - nf:512], selt_sb[:],
                    rc2[:].rearrange("p a b c -> p (a b c)"),
                    start=True, stop=True, skip_group_check=True)
                bc = small.tile([P, 2, CO, B], F32, tag="bc")
                nc.vector.tensor_copy(
                    bc[:].rearrange("p a b c -> p (a b c)"),
                    pt1[:, 512 - nf:512])
                # a = rstd*gnw, c = (-mean*rstd)*gnw + gnb, per batch
                for b in range(B):
                    nc.vector.tensor_tensor(a_sb[:, :, b], bc[:, 0, :, b],
                                            gnw_sb[:], ALU.mult)
                    nc.vector.tensor_tensor(c_sb[:, :, b], bc[:, 1, :, b],
                                            gnw_sb[:], ALU.mult)
                    nc.vector.tensor_tensor(c_sb[:, :, b], c_sb[:, :, b],
                                            gnb_sb[:], ALU.add)

            # ---- per-batch: correction vectors via PE ----
            def vec_prep(b, ps_pool):
                pv = small.tile([P, CO, 2], F32R, tag="pv")
                for co in range(CO):
                    nc.vector.tensor_tensor(pv[:, co, 0:1], a_sb[:, co, b:b+1],
                                            stg[:, 0, co, b:b + 1], ALU.mult)
                    nc.vector.tensor_copy(pv[:, co, 1:2], c_sb[:, co, b:b+1])
                # u and W@c rows accumulate in separate PSUM banks, both
                # at partition 0 (single-partition ops must sit at base 0)
                for half in range(2):
                    vpu = ps_pool.tile([P, C], F32, tag="yl",
                                       name=f"vecu{b}_{half}")
                    vpc = ps_pool.tile([P, C], F32, tag="yl",
                                       name=f"vecc{b}_{half}")
                    sl = slice(half * 512, half * 512 + 512)
                    for co in range(CO):
                        nc.tensor.matmul(
                            vpu[0:1, 0:C], pv[:, co, 0:1], wqk_sb[:, co, sl],
                            start=(co == 0), stop=(co == CO - 1),
                            skip_group_check=True)
                        nc.tensor.matmul(
                            vpc[0:1, 0:C], pv[:, co, 1:2], wqk_sb[:, co, sl],
                            start=(co == 0), stop=(co == CO - 1),
                            skip_group_check=True)
                    # u = A@Sx -> uqk8; bqk = W@c + qkv bias (from PSUM)
                    nc.vector.tensor_scalar_mul(uqk8_l[b][:, sl],
                                                vpu[0:1, :], 1.0 / N_CORES)
                    nc.vector.tensor_tensor(bqk_l[b][:, sl], vpc[0:1, :],
                                            qkb_sb[:, sl], ALU.add)
                # nbq8 = N*bqk/8
                nc.vector.tensor_scalar_mul(nbq8_l[b][:], bqk_l[b][:],
                                            float(ntot) / N_CORES)
                if debug:
                    nc.sync.dma_start(dbg["dbg_vec"][:, b], ub[:])

            # ---- per-batch: Y = (a.Gx) Wk^T, Y' = a.Y, L = Wq^T Y' ----
            # both diag(a) GN scales fold into the evacuations: an in-place
            # DVE row-scale of the Gram and an ACT scale-copy of Y
            def y_l(b, gsb):
                for co in range(CO):
                    nc.vector.tensor_scalar_mul(gsb[:, co], gsb[:, co],
                                                a_sb[:, co, b:b + 1])
                y_sb = ysb.tile([P, CO, C], F32R, tag="y", name=f"ysb{b}")
                for c1 in range(CO):
                    yp = ylps.tile([P, C], F32, tag="yl", name=f"y{b}_{c1}")
                    for c2 in range(CO):
                        nc.tensor.matmul(
                            yp[:], gsb[:, c2, c1 * P:(c1 + 1) * P],
                            wqk_sb[:, c2, C:2 * C],
                            start=(c2 == 0), stop=(c2 == CO - 1))
                    if c1 % 2 == 0:
                        nc.scalar.activation(y_sb[:, c1], yp[:],
                                             ACT.Identity,
                                             scale=a_sb[:, c1, b:b + 1])
                    else:
                        nc.vector.tensor_scalar_mul(y_sb[:, c1], yp[:],
                                                    a_sb[:, c1, b:b + 1])
                last_mm = None
                for dc in range(CO):
                    lp = ylps.tile([P, C], F32, tag="yl", name=f"l{b}_{dc}")
                    for c1 in range(CO):
                        nc.tensor.matmul(
                            lp[:], wqk_sb[:, c1, dc * P:(dc + 1) * P],
                            y_sb[:, c1], start=(c1 == 0), stop=False,
                            skip_group_check=True)
                    # rank-1 corrections on the two diagonal head blocks
                    for par in range(2):
                        hh = 2 * dc + par
                        rows = slice(par * 64, par * 64 + 64)
                        cols = slice(hh * 64, hh * 64 + 64)
                        tp = (0, 64) if par else None
                        ksl = slice(C + hh * 64, C + hh * 64 + 64)
                        qsl = slice(hh * 64, hh * 64 + 64)
                        nc.tensor.matmul(
                            lp[rows, cols], uqk8_l[b][:, qsl],
                            bqk_l[b][:, ksl], start=False, stop=False,
                            tile_position=tp, skip_group_check=True)
                        nc.tensor.matmul(
                            lp[rows, cols], bqk_l[b][:, qsl],
                            uqk8_l[b][:, ksl], start=False, stop=False,
                            tile_position=tp, skip_group_check=True)
                        last_mm = nc.tensor.matmul(
                            lp[rows, cols], nbq8_l[b][:, qsl],
                            bqk_l[b][:, ksl], start=False, stop=(par == 1),
                            tile_position=tp, skip_group_check=True)
                    # extract diagonal head blocks (alternate DVE/ACT to
                    # halve the serialized per-op SEQ latency chain)
                    for par in range(2):
                        hh = 2 * dc + par
                        rows = slice(par * 64, par * 64 + 64)
                        if par == 0:
                            nc.scalar.copy(
                                logits_sb[rows, b, dc, :],
                                lp[rows, hh * 64:hh * 64 + 64])
                        else:
                            nc.vector.tensor_copy(
                                logits_sb[rows, b, dc, :],
                                lp[rows, hh * 64:hh * 64 + 64])
                return last_mm

            def lg_ar_in(p):
                # paired logits AllReduce: batches 2p, 2p+1 in one 256KB op
                nc.gpsimd.dma_start(lg_in_l[p][:],
                                    logits_sb[:, 2 * p:2 * p + 2])
                nc.gpsimd.collective_compute(
                    "AllReduce", ALU.add,
                    replica_groups=[list(range(N_CORES))],
                    ins=[lg_in_l[p].opt()], outs=[lg_out_l[p].opt()],
                )

            def lg_ar_out(p):
                # emitted after BOTH collectives so the copy-back of pair 0
                # does not block pair 1's input DMA in the Pool FIFO
                nc.gpsimd.dma_start(lg_full[:, 2 * p:2 * p + 2],
                                    lg_out_l[p][:])

            # ============ emit phase A/B in PE-schedule order ============
            # stats feed: all batches' bn_stats read the transpose tiles
            # directly — DMA transfers serialize on the DMA engines in any
            # case, so extra stats loads only delay the collective
            for b in range(B):
                load_t(b, stats_here=True)
            nc.gpsimd.dma_start(qkb_sb[:], qkb[:])
            nc.gpsimd.dma_start(vb_sb[:], vb2[:])
            nc.gpsimd.dma_start(ob_sb[:], ob2[:])
            nc.gpsimd.dma_start(gnw_sb[:], gnw2[:])
            nc.gpsimd.dma_start(gnb_sb[:], gnb2[:])
            # weights BEFORE st_in in the Pool FIFO: st_in waits for stats
            # anyway, and anything after it would delay the logits ARs
            wqk_sb = wqp.tile([P, CO, 2 * C], F32R)
            nc.gpsimd.dma_start(
                wqk_sb[:], wqk_t.rearrange("(co ci) o -> ci co o", ci=P))
            wv_sb = wvp.tile([P, CO, C], F32R)
            wo_sb = wvp.tile([P, CO, C], F32R)
            nc.gpsimd.dma_start(
                wv_sb[:], wv_t.rearrange("(co ci) o -> ci co o", ci=P))
            nc.gpsimd.dma_start(
                wo_sb[:], wo_t.rearrange("(co ci) o -> ci co o", ci=P))
            stats_ar()

            t_gx_run([0, 1, 2])
            # most of batch 3's Gram fills the PE while the stats
            # collective completes; the tail resumes after YL0/YL1
            T3A = 3 * NT // 4
            t_gx_run(None, steps=[(3, tb) for tb in range(T3A)])
            prep_all(ylps)
            vec_prep(0, ylps)
            vec_prep(1, ylps)
            y_l(0, gsb_l.pop(0))
            y_l(1, gsb_l.pop(1))
            lg_ar_in(0)
            vec_prep(2, ylps)
            vec_prep(3, ylps)
            t_gx_run(None, steps=[(3, tb) for tb in range(T3A, NT)])
            y_l(2, gsb_l.pop(2))
            y_l(3, gsb_l.pop(3))
            lg_ar_in(1)   # pair-1 input DMA precedes pair-0's copy-back in
            lg_ar_out(0)  # the Pool FIFO so AR23 starts the moment AR01 ends
            lg_ar_out(1)
            _ = gh  # silence lint; gh used by load_t

            if debug:
                nc.sync.dma_start(dbg["dbg_a"][:], a_sb[:])
                nc.sync.dma_start(dbg["dbg_c"][:], c_sb[:])
                nc.sync.dma_start(dbg["dbg_stats"][:], stg[:])
                nc.sync.dma_start(dbg["dbg_logits"][:], lg_full[:])
            ab_stack.close()

            # ================= phase C =================
            with (
                tc.tile_pool(name="cpers", bufs=1) as cpers,
                tc.tile_pool(name="xc", bufs=5) as xc,
                tc.tile_pool(name="hp", bufs=2) as hp,
                tc.tile_pool(name="vp", bufs=4) as vp,
                tc.tile_pool(name="avp", bufs=2) as avp,
                tc.tile_pool(name="yp", bufs=2) as yp,
                tc.tile_pool(name="smp", bufs=4) as smp,
                tc.tile_pool(name="cps", bufs=6, space="PSUM") as cps,
                tc.tile_pool(name="trp", bufs=2, space="PSUM") as trp,
            ):
                attn_sb = cpers.tile([P, B, 4, D], F32)
                abd_f = cpers.tile([P, B * 4, P], F32)   # block-diag attn
                abd_r = cpers.tile([P, B * 4, P], F32R)  # transposed, f32r
                nc.vector.memset(abd_f[:], 0.0)
                # transposed out-bias rows + ones row: the out bias is
                # added by a rank-1 PE matmul folded into the out-proj
                obt4 = cpers.tile([1, CO, P], mybir.dt.bfloat16)
                ones4 = cpers.tile([1, NC], mybir.dt.bfloat16)
                nc.vector.memset(ones4[:], 1.0)
                for ot in range(CO):
                    pt = trp.tile([P, P], F32, tag="pt", name=f"ob{ot}")
                    nc.tensor.transpose(pt[0:1, :], ob_sb[:, ot:ot + 1],
                                        ident[:])
                    nc.scalar.copy(obt4[:, ot], pt[0:1, :])

                def softmax_b(b):
                    for hp_i in range(4):
                        blk = lg_full[:, b, hp_i]
                        mx = smp.tile([P, 1], F32, tag="mx")
                        nc.vector.reduce_max(mx[:], blk, AX)
                        nbias = smp.tile([P, 1], F32, tag="nb")
                        nc.vector.tensor_scalar_mul(nbias[:], mx[:], -scale)
                        ex = attn_sb[:, b, hp_i]
                        nc.scalar.activation(ex, blk, ACT.Exp, bias=nbias[:],
                                             scale=scale)
                        sm = smp.tile([P, 1], F32, tag="sm")
                        nc.vector.reduce_sum(sm[:], ex, AX)
                        nc.vector.reciprocal(sm[:], sm[:])
                        nc.vector.tensor_scalar_mul(ex, ex, sm[:])
                        idx = b * 4 + hp_i
                        nc.vector.tensor_copy(abd_f[0:64, idx, 0:64],
                                              attn_sb[0:64, b, hp_i])
                        nc.vector.tensor_copy(abd_f[64:128, idx, 64:128],
                                              attn_sb[64:128, b, hp_i])

                def tr_attn(b):
                    for hp_i in range(4):
                        idx = b * 4 + hp_i
                        pt = trp.tile([P, P], F32, tag="pt")
                        nc.tensor.transpose(pt[:], abd_f[:, idx, :], ident[:])
                        nc.scalar.copy(abd_r[:, idx], pt[:])

                def emit_v(b, j):
                    xv = xin[b].rearrange("(co ci) n -> ci co n", ci=P)
                    xa = xc.tile([P, CO, NC], F32, tag="x")
                    eng = nc.sync if (j % 2 == 0) else nc.scalar
                    eng.dma_start(xa[:], xv[:, :, j * NC:(j + 1) * NC])
                    h = hp.tile([P, CO, NC], F32R, tag="h")
                    for co in range(CO):
                        nc.vector.tensor_scalar(
                            h[:, co], xa[:, co],
                            a_sb[:, co, b:b + 1], c_sb[:, co, b:b + 1],
                            ALU.mult, ALU.add)
                    v = vp.tile([P, CO, NC], F32R, tag="v")
                    for ot in range(CO):
                        ps_v = cps.tile([P, NC], F32, tag="c")
                        for co in range(CO):
                            nc.tensor.matmul(
                                ps_v[:], wv_sb[:, co, ot * P:(ot + 1) * P],
                                h[:, co], start=(co == 0), stop=(co == CO - 1))
                        if ot % 2 == 0:
                            nc.scalar.activation(v[:, ot], ps_v[:],
                                                 ACT.Identity,
                                                 bias=vb_sb[:, ot:ot + 1])
                        else:
                            nc.vector.tensor_scalar_add(
                                v[:, ot], ps_v[:], vb_sb[:, ot:ot + 1])
                    if debug and b == 0 and j == 0:
                        nc.gpsimd.dma_start(dbg["dbg_v"][:], v[:])
                    return xa, v

                def finish(b, j, xa, v):
                    av = avp.tile([P, CO, NC], F32R, tag="av")
                    for ot in range(CO):
                        ps_a = cps.tile([P, NC], F32, tag="c")
                        nc.tensor.matmul(ps_a[:], abd_r[:, b * 4 + ot],
                                         v[:, ot], start=True, stop=True)
                        if ot % 2 == 0:
                            nc.scalar.copy(av[:, ot], ps_a[:])
                        else:
                            nc.vector.tensor_copy(av[:, ot], ps_a[:])
                    if debug and b == 0 and j == 0:
                        nc.gpsimd.dma_start(dbg["dbg_av"][:], av[:])
                    yv = yout[b].rearrange("(co ci) n -> ci co n", ci=P)
                    y_sb = yp.tile([P, CO, NC], F32, tag="y")
                    for ot in range(CO):
                        ps_o = cps.tile([P, NC], F32, tag="c")
                        for co in range(CO):
                            nc.tensor.matmul(
                                ps_o[:], wo_sb[:, co, ot * P:(ot + 1) * P],
                                av[:, co], start=(co == 0), stop=False)
                        # out bias as a rank-1 accumulation, then +residual
                        nc.tensor.matmul(ps_o[:], obt4[:, ot], ones4[:],
                                         start=False, stop=True,
                                         skip_group_check=True)
                        nc.vector.tensor_tensor(
                            y_sb[:, ot], ps_o[:], xa[:, ot], ALU.add)
                    eng = nc.sync if (j % 2 == 0) else nc.scalar
                    eng.dma_start(yv[:, :, j * NC:(j + 1) * NC], y_sb[:])

                softmax_b(0)
                for b in range(B):
                    pend = []
                    for j in range(nchunks):
                        pend.append((j, *emit_v(b, j)))
                    tr_attn(b)
                    if b + 1 < B:
                        # prefetch next batch's softmax so its attn
                        # transposes don't stall the PE
                        softmax_b(b + 1)
                    for j, xa, v in pend:
                        finish(b, j, xa, v)
                if debug:
                    nc.sync.dma_start(dbg["dbg_attn"][:], attn_sb[:])

    return nc


_WAITSPLIT_COUNTER = [0]


def _split_waits(nc, limit: int = 1):
    """Walrus in this container rejects instructions with more than one sync
    wait; split extras onto injected NoOps on the same engine."""
    n_split = 0
    for fn in nc.m.functions:
        for bb in fn.blocks:
            insts = list(bb.instructions)
            out = []
            changed = False
            for inst in insts:
                si = inst.sync_info
                waits = list(si.on_wait) if si is not None and si.on_wait \
                    else []
                if len(waits) > limit:
                    keep = waits[-limit:]
                    extra = waits[:-limit]
                    for i in range(0, len(extra), limit):
                        chunk = extra[i:i + limit]
                        _WAITSPLIT_COUNTER[0] += 1
                        nop = mybir.InstNoOp(
                            name=f"waitsplit-{_WAITSPLIT_COUNTER[0]}",
                            ins=[], outs=[])
                        nop.engine = inst.engine
                        nop.sync_info = mybir.SyncInfo(
                            on_wait=chunk, on_update=[])
                        out.append(nop)
                    si.on_wait = keep
                    n_split += 1
                    changed = True
                out.append(inst)
            if changed:
                bb.instructions = out
    return n_split


_CACHE = {}


def _get_module(n_loc, split=True, debug=False):
    key = (n_loc, split, debug)
    if key not in _CACHE:
        nc = build_module(n_loc, debug=debug)
        if split:
            _split_waits(nc, limit=1)
        _CACHE[key] = nc
    return _CACHE[key]


def make_in_maps(inputs, n_loc=None):
    x = np.ascontiguousarray(np.asarray(inputs["x"], dtype=np.float32))
    qkv_w = np.asarray(inputs["qkv_w"], dtype=np.float32)
    qkv_b = np.asarray(inputs["qkv_b"], dtype=np.float32)
    out_w = np.asarray(inputs["out_w"], dtype=np.float32)
    out_b = np.asarray(inputs["out_b"], dtype=np.float32)
    gn_w = np.asarray(inputs["gn_weight"], dtype=np.float32)
    gn_b = np.asarray(inputs["gn_bias"], dtype=np.float32)

    n_tot = int(np.prod(x.shape[2:]))
    if n_loc is None:
        n_loc = n_tot // N_CORES
    xf = x.reshape(B, C, n_tot)

    wqk_t = np.ascontiguousarray(_round_tf32(qkv_w[0:2 * C].T))
    wv_t = np.ascontiguousarray(_round_tf32(qkv_w[2 * C:3 * C].T))
    wo_t = np.ascontiguousarray(_round_tf32(out_w.T))
    qkb = np.ascontiguousarray(qkv_b[0:2 * C].reshape(1, 2 * C))
    vb2 = np.ascontiguousarray(qkv_b[2 * C:3 * C].reshape(CO, P).T)
    ob2 = np.ascontiguousarray(out_b.reshape(CO, P).T)
    gnw2 = np.ascontiguousarray(gn_w.reshape(CO, P).T)
    gnb2 = np.ascontiguousarray(gn_b.reshape(CO, P).T)

    shared = dict(wqk_t=wqk_t, wv_t=wv_t, wo_t=wo_t, qkb=qkb, vb2=vb2,
                  ob2=ob2, gnw2=gnw2, gnb2=gnb2)
    in_maps = []
    for c in range(N_CORES):
        sl = np.ascontiguousarray(xf[:, :, c * n_loc:(c + 1) * n_loc])
        in_maps.append({"xin": sl, **shared})
    return in_maps


def run(inputs, n_loc=None, debug=False, **kw):
    x = np.asarray(inputs["x"])
    n_tot = int(np.prod(x.shape[2:]))
    if n_loc is None:
        n_loc = n_tot // N_CORES
    nc = _get_module(n_loc, debug=debug)
    in_maps = make_in_maps(inputs, n_loc)
    res = bass_utils.run_bass_kernel_spmd(
        nc, in_maps, core_ids=list(range(N_CORES)), **kw)
    y = np.concatenate([res.results[c]["yout"] for c in range(N_CORES)],
                       axis=2)
    return y, res


def kernel(**inputs) -> np.ndarray:
    x = np.asarray(inputs["x"])
    y, _ = run(inputs)
    return y.reshape(x.shape).astype(np.asarray(x).dtype)
